# revision 10
# baseline (speedup 1.0000x reference)
"""CrossAttention kernel for 8 Trainium2 NeuronCores.

Sharding: batch (4) x query-row-half (2) -> 8 shards, one per core. Each core
computes the full cross-attention for its 1024 query rows of one batch:
Q/K/V projections, 8 heads of attention, and the output projection. K/V
projections are recomputed by both cores sharing a batch (20% extra flops)
in exchange for zero collectives and a pure-SPMD single NEFF.

Layout trick: x and context are transposed (and cast to bf16) on the host so
the contraction dim lands on SBUF partitions with contiguous DMAs; all device
matmuls run without on-chip transposes:
  QT = Wq.T @ xT      (i on partitions)     KT = Wk.T @ ctxT
  V  = ctxT.T @ Wv    (natural [nk, i])
  ST_h = KT_h @ QT_h  ([nk, nq], K=64, head pairs in PE row groups)
  P = exp(ST * scale) (no max-subtraction; logits are ~N(0,1), safe range)
  O^T_h | den_h = [V_h | ones].T @ P  (denominator rides free in the M dim)
  Y = (O^T/den).T @ Wo + bo

Schedule (v4): engine queues execute in order, so the emission order IS the
schedule. Each of a step's 8 score groups is followed immediately by PV
matmuls of the previous step's pair (head0 2/slot in slots 0-3, head1
3/3/2 in slots 5-7) plus at most ~1 projection unit, so the ACT engine's
exp stream never waits more than a slot. The softmax denominator chain
(stage den row -> chop DMA -> reciprocal -> scatter DMA) launches per head
as soon as that head's accumulation stops; the 1/den broadcast + normalize
runs at slot 4 two steps later, which also bounds PV-accumulator lifetime
so 3 PSUM banks suffice (scores 4 + PV 3 + scratch 1 = 8). Input DMAs are
chunked in first-consumption order across the three queues (~100GB/s
each): scalar carries the small weight tiles and finishes before the
first exp, sync and gpsimd carry the bulk. V-pad ones blocks are memset
on gpsimd instead of DMAed. The final pair's head0 PV interleaves into
the last step and the last two normalizations overlap the split-k output
projection drain, so only ~10us trails the last exp. Y is stored bf16
(upcast on host) to halve the output DMA.
"""

import numpy as np

HEADS = 8
DIM_HEAD = 64
SCALE = DIM_HEAD ** -0.5
B, NQ, DQ = 4, 2048, 512
NK, DC = 1024, 768
INNER = HEADS * DIM_HEAD  # 512
NQH = NQ // 2             # query rows per core
N_CORES = 8
P = 128

_PROG_CACHE = {}


def _build_program():
    import concourse.bacc as bacc
    import concourse.tile as tile
    from concourse import mybir
    from concourse.bass import ts, ds

    f32 = mybir.dt.float32
    f32r = mybir.dt.float32r
    bf16 = mybir.dt.bfloat16
    Exp = mybir.ActivationFunctionType.Exp

    nc = bacc.Bacc(
        "TRN2",
        target_bir_lowering=False,
        debug=False,
        num_devices=N_CORES,
    )

    KQ = DQ // P      # 4  k-tiles for x-side contraction
    KC = DC // P      # 6  k-tiles for context-side contraction
    KI = INNER // P   # 4  k-tiles for inner-dim contraction
    NQT = NQH // P    # 8  query row tiles
    NKT = NK // P     # 8  key row tiles
    NCH = NQH // 512  # 2  nq chunks of 512

    # Wq/Wk host-tiled [m, p, ko, 128] so per-m loads are contiguous.
    xT_d = nc.dram_tensor("xT", [DQ, NQH], bf16, kind="ExternalInput")
    ctxT_d = nc.dram_tensor("ctxT", [DC, NK], bf16, kind="ExternalInput")
    Wq_d = nc.dram_tensor("Wq", [KQ, P, KQ, P], bf16, kind="ExternalInput")
    Wk_d = nc.dram_tensor("Wk", [KQ, P, KC, P], bf16, kind="ExternalInput")
    Wv_d = nc.dram_tensor("Wv", [P, KC, INNER], bf16, kind="ExternalInput")
    Wo_d = nc.dram_tensor("Wo", [P, KI, DQ], bf16, kind="ExternalInput")
    bo_d = nc.dram_tensor("bo", [DQ], f32, kind="ExternalInput")
    diag_d = nc.dram_tensor("diag2", [P, 128], f32r, kind="ExternalInput")
    zr_d = nc.dram_tensor("zr", [P, 512], f32r, kind="ExternalInput")
    Y_d = nc.dram_tensor("Y", [NQH, DQ], bf16, kind="ExternalOutput")

    with tile.TileContext(nc) as tc:
        with (
            tc.tile_pool(name="consts", bufs=1) as consts,
            tc.tile_pool(name="sc", bufs=2, space="PSUM") as scp,
            tc.tile_pool(name="pv", bufs=3, space="PSUM") as pvp,
            tc.tile_pool(name="mm", bufs=1, space="PSUM") as mmp,
            tc.tile_pool(name="ep", bufs=16) as ep,
            tc.tile_pool(name="dn", bufs=2) as dnp,
            tc.tile_pool(name="yp", bufs=2) as yp,
        ):
            Wk_sb = consts.tile([P, KQ, KC, P], bf16, tag="wk")
            Wq_sb = consts.tile([P, KQ, KQ, P], bf16, tag="wq")
            Wv_sb = consts.tile([P, KC, INNER], bf16, tag="wv")
            Wo_sb = consts.tile([P, KI, DQ], bf16, tag="wo")
            ctx_sb = consts.tile([P, KC, NK], bf16, tag="ctx")
            xT_sb = consts.tile([P, KQ, NQH], bf16, tag="x")
            bo_sb = consts.tile([P, DQ], f32, tag="bo")
            diag_sb = consts.tile([P, P], f32r, tag="diag")
            # two independent den-chain buffer sets alternating per step so
            # consecutive normalizations never serialize on WAR hazards.
            # r rows != {0, 64} must read as exact zeros for the mask-matmul
            # broadcast (0 * garbage-NaN would poison it); memset can't
            # write f32r, so clear via DMA from a zero dram tensor (r_a)
            # and an on-chip copy (r_b).
            r_a = consts.tile([P, 512], f32r, tag="ra")
            r_b = consts.tile([P, 512], f32r, tag="rb2")
            d_a = consts.tile([P, 512], f32, tag="da")
            d_b = consts.tile([P, 512], f32, tag="db")

            ctx_src = ctxT_d.ap().rearrange("(ko p) n -> p ko n", p=P)
            xT_src = xT_d.ap().rearrange("(ko p) n -> p ko n", p=P)

            # ---- input DMA emission, in first-consumption order ----------
            # Each queue sustains ~60-100GB/s; the issue (~0.7us engine
            # time) is what occupies the engine, transfers stream behind.
            # scalar carries only the small weight tiles (done early, engine
            # free for exp); sync and gpsimd carry the bulk.
            nc.scalar.dma_start(out=Wk_sb[:, 0], in_=Wk_d.ap()[0])
            nc.sync.dma_start(
                out=ctx_sb[:, 0:3, 0:512], in_=ctx_src[:, 0:3, 0:512]
            )
            nc.gpsimd.dma_start(
                out=xT_sb[:, :, 0:512], in_=xT_src[:, :, 0:512]
            )
            nc.scalar.dma_start(out=Wq_sb[:, 0], in_=Wq_d.ap()[0])
            nc.sync.dma_start(
                out=ctx_sb[:, 3:6, 0:512], in_=ctx_src[:, 3:6, 0:512]
            )
            nc.scalar.dma_start(out=Wk_sb[:, 1], in_=Wk_d.ap()[1])
            nc.gpsimd.dma_start(
                out=ctx_sb[:, 0:3, 512:1024], in_=ctx_src[:, 0:3, 512:1024]
            )
            nc.scalar.dma_start(out=Wq_sb[:, 1], in_=Wq_d.ap()[1])
            nc.gpsimd.dma_start(
                out=ctx_sb[:, 3:6, 512:1024], in_=ctx_src[:, 3:6, 512:1024]
            )
            nc.scalar.dma_start(
                out=Wv_sb[:, 0:3, :], in_=Wv_d.ap()[:, 0:3, :]
            )
            nc.sync.dma_start(
                out=Wv_sb[:, 3:6, :], in_=Wv_d.ap()[:, 3:6, :]
            )
            nc.scalar.dma_start(out=Wk_sb[:, 2], in_=Wk_d.ap()[2])
            nc.scalar.dma_start(out=Wq_sb[:, 2], in_=Wq_d.ap()[2])
            nc.scalar.dma_start(out=Wk_sb[:, 3], in_=Wk_d.ap()[3])
            nc.scalar.dma_start(out=Wq_sb[:, 3], in_=Wq_d.ap()[3])
            nc.gpsimd.dma_start(out=diag_sb, in_=diag_d.ap())
            nc.gpsimd.dma_start(out=r_a, in_=zr_d.ap())
            nc.sync.dma_start(
                out=xT_sb[:, :, 512:1024], in_=xT_src[:, :, 512:1024]
            )
            nc.sync.dma_start(out=Wo_sb, in_=Wo_d.ap())
            nc.gpsimd.dma_start(
                out=bo_sb, in_=bo_d.ap().unsqueeze(0).to_broadcast((P, DQ))
            )

            KT_sb = consts.tile([P, KI, NK], bf16, tag="kt")    # [i, nk]
            QT_sb = consts.tile([P, KI, NQH], bf16, tag="qt")   # [i, nq]
            # V in natural [nk, i] layout padded per head to 128 cols:
            # even head h: cols h*128+[0:64]=V_h, [64:128]=ones
            # odd  head h: cols h*128+[0:64]=ones, [64:128]=V_h
            V_sb = consts.tile([P, NKT, HEADS * P], bf16, tag="v")
            OT_sb = consts.tile([P, KI, NQH], bf16, tag="ot")   # [i, nq]

            for t in range(NKT):
                dv4 = V_sb[:, t, :].rearrange("p (j y) -> p j y", j=4)
                nc.gpsimd.memset(dv4[:, :, 64:192], 1.0)

            # ---- PE work units (emitted as schedule filler) ----
            def kp_unit(m, c):  # K projection: KT[:, m, c*512:...]
                psk = mmp.tile([P, 512], f32, tag="acc")
                for k in range(KC):
                    nc.tensor.matmul(
                        psk,
                        lhsT=Wk_sb[:, m, k, :],
                        rhs=ctx_sb[:, k, ds(c * 512, 512)],
                        start=(k == 0),
                        stop=(k == KC - 1),
                    )
                nc.vector.tensor_copy(KT_sb[:, m, ds(c * 512, 512)], psk)

            def qp_unit(m, c):  # Q projection: QT[:, m, c*512:...]
                psq = mmp.tile([P, 512], f32, tag="acc")
                for k in range(KQ):
                    nc.tensor.matmul(
                        psq,
                        lhsT=Wq_sb[:, m, k, :],
                        rhs=xT_sb[:, k, ds(c * 512, 512)],
                        start=(k == 0),
                        stop=(k == KQ - 1),
                    )
                nc.vector.tensor_copy(QT_sb[:, m, ds(c * 512, 512)], psq)

            def vp_unit(t):  # V projection tile t, scattered into head pads
                psv = mmp.tile([P, 512], f32, tag="acc")
                for k in range(KC):
                    nc.tensor.matmul(
                        psv,
                        lhsT=ctx_sb[:, k, ts(t, P)],
                        rhs=Wv_sb[:, k, :],
                        start=(k == 0),
                        stop=(k == KC - 1),
                    )
                pv4 = psv.rearrange("p (j x) -> p j x", j=4)
                dv4 = V_sb[:, t, :].rearrange("p (j y) -> p j y", j=4)
                nc.vector.tensor_copy(dv4[:, :, 0:64], pv4[:, :, 0:64])
                nc.vector.tensor_copy(dv4[:, :, 192:256], pv4[:, :, 64:128])

            def op_unit(m):  # output projection row tile m (full k)
                psy = mmp.tile([P, 512], f32, tag="acc")
                for k in range(KI):
                    nc.tensor.matmul(
                        psy,
                        lhsT=OT_sb[:, k, ts(m, P)],
                        rhs=Wo_sb[:, k, :],
                        start=(k == 0),
                        stop=(k == KI - 1),
                    )
                y_t = yp.tile([P, DQ], bf16, tag="y")
                nc.vector.tensor_tensor(y_t, psy, bo_sb, op=mybir.AluOpType.add)
                eng = nc.sync if m % 2 == 0 else nc.gpsimd
                eng.dma_start(out=Y_d.ap()[ts(m, P), :], in_=y_t)

            # split-k output projection for the last 4 row tiles: k=0..1 run
            # during the final step, k=2/k=3 drain between the last two
            # normalizations at the tail
            partials = {}

            def opk01_unit(m):
                psy = mmp.tile([P, 512], f32, tag="acc")
                for k in range(2):
                    nc.tensor.matmul(
                        psy,
                        lhsT=OT_sb[:, k, ts(m, P)],
                        rhs=Wo_sb[:, k, :],
                        start=(k == 0),
                        stop=(k == 1),
                    )
                part = yp.tile([P, 512], f32, tag="part", bufs=4)
                nc.vector.tensor_tensor(
                    part, psy, bo_sb, op=mybir.AluOpType.add
                )
                partials[m] = part

            def opk2_unit(m):
                psy = mmp.tile([P, 512], f32, tag="acc")
                nc.tensor.matmul(
                    psy, lhsT=OT_sb[:, 2, ts(m, P)], rhs=Wo_sb[:, 2, :],
                    start=True, stop=True,
                )
                part2 = yp.tile([P, 512], f32, tag="part2", bufs=4)
                nc.vector.tensor_tensor(
                    part2, psy, partials[m], op=mybir.AluOpType.add
                )
                partials[m] = part2

            def opk3_unit(m):
                psy = mmp.tile([P, 512], f32, tag="acc")
                nc.tensor.matmul(
                    psy, lhsT=OT_sb[:, 3, ts(m, P)], rhs=Wo_sb[:, 3, :],
                    start=True, stop=True,
                )
                y_t = yp.tile([P, DQ], bf16, tag="y")
                nc.vector.tensor_tensor(
                    y_t, psy, partials[m], op=mybir.AluOpType.add
                )
                eng = nc.sync if m % 2 == 0 else nc.gpsimd
                eng.dma_start(out=Y_d.ap()[ts(m, P), :], in_=y_t)

            def s_group(j, c, t):  # one nk-tile of scores for head pair j
                ps_g = scp.tile([P, 2, 512], f32, tag="s")
                e_g = ep.tile([P, 2, 512], bf16, tag="e")
                nc.tensor.matmul(
                    ps_g[:, 0, :],
                    lhsT=KT_sb[0:64, j, ts(t, P)],
                    rhs=QT_sb[0:64, j, ds(c * 512, 512)],
                    start=True, stop=True,
                )
                nc.tensor.matmul(
                    ps_g[:, 1, :],
                    lhsT=KT_sb[64:128, j, ts(t, P)],
                    rhs=QT_sb[64:128, j, ds(c * 512, 512)],
                    start=True, stop=True,
                )
                nc.scalar.activation(out=e_g, in_=ps_g, func=Exp, scale=SCALE)
                return e_g

            # ---- PV of a pair, emitted in per-slot chunks ---------------
            def new_pv(j, c, e_gs, parity):
                return {"j": j, "c": c, "e": e_gs, "par": parity,
                        "po": [None, None], "dr": None}

            def pv_emit(st, ab, tts):
                """PV matmuls for head `2j+ab` over nk tiles tts; when the
                accumulation stops (tt==7), stage that head's den row and
                launch its half of the reciprocal chain."""
                d_sb = d_a if st["par"] == 0 else d_b
                r_sb = r_a if st["par"] == 0 else r_b
                h = 2 * st["j"] + ab
                if st["po"][ab] is None:
                    st["po"][ab] = pvp.tile(
                        [P, 512], f32, tag="po", name=f"po{ab}"
                    )
                po = st["po"][ab]
                for tt in tts:
                    nc.tensor.matmul(
                        po,
                        lhsT=V_sb[:, tt, ds(h * P, P)],
                        rhs=st["e"][tt][:, ab, :],
                        start=(tt == 0),
                        stop=(tt == NKT - 1),
                    )
                if tts[-1] == NKT - 1:
                    if ab == 0:
                        nc.vector.tensor_copy(d_sb[64:65, :], po[64:65, :])
                        st["dr"] = dnp.tile(
                            [64, 16], f32, tag="dr", name="dr"
                        )
                        nc.gpsimd.dma_start(
                            out=st["dr"][:, 0:8], in_=d_sb[64:65, :]
                        )
                    else:
                        dr = st["dr"]
                        nc.vector.tensor_copy(d_sb[0:1, :], po[0:1, :])
                        nc.gpsimd.dma_start(
                            out=dr[:, 8:16], in_=d_sb[0:1, :]
                        )
                        rr = dnp.tile([64, 16], f32r, tag="rr")
                        with nc.allow_low_precision(
                            reason="1/den feeds an f32r matmul"
                        ):
                            nc.vector.reciprocal(rr, dr)
                        nc.gpsimd.dma_start(
                            out=r_sb[64:65, :], in_=rr[:, 0:8]
                        )
                        nc.gpsimd.dma_start(
                            out=r_sb[0:1, :], in_=rr[:, 8:16]
                        )

            def make_finish(st):
                r_sb = r_a if st["par"] == 0 else r_b
                j, c, pos = st["j"], st["c"], st["po"]

                def finish():
                    ps_rb = mmp.tile([P, 512], f32, tag="acc")
                    nc.tensor.matmul(
                        ps_rb, lhsT=diag_sb, rhs=r_sb, start=True, stop=True
                    )
                    # DVE may read only one PSUM operand per instruction
                    rb_sb = yp.tile([P, 512], f32, tag="rb")
                    nc.vector.tensor_copy(rb_sb, ps_rb)
                    csl = ds(c * 512, 512)
                    nc.vector.tensor_tensor(
                        OT_sb[0:64, j, csl], pos[0][0:64, :], rb_sb[0:64, :],
                        op=mybir.AluOpType.mult,
                    )
                    nc.vector.tensor_tensor(
                        OT_sb[64:128, j, csl], pos[1][64:128, :],
                        rb_sb[64:128, :], op=mybir.AluOpType.mult,
                    )
                return finish

            # per-step slot fillers: {step: {slot: [unit, ...]}}
            sched = {
                0: {2: [lambda: vp_unit(0)],
                    3: [lambda: kp_unit(0, 1), lambda: vp_unit(1)],
                    4: [lambda: vp_unit(2)],
                    5: [lambda: vp_unit(3), lambda: kp_unit(1, 0)],
                    6: [lambda: vp_unit(4), lambda: qp_unit(1, 0)],
                    7: [lambda: vp_unit(5), lambda: kp_unit(1, 1)]},
                1: {0: [lambda: vp_unit(6)],
                    1: [lambda: vp_unit(7)],
                    4: [lambda: kp_unit(2, 0)],
                    6: [lambda: kp_unit(2, 1)],
                    7: [lambda: qp_unit(2, 0)]},
                2: {0: [lambda: nc.vector.tensor_copy(r_b, r_a)],
                    1: [lambda: kp_unit(3, 0)],
                    3: [lambda: kp_unit(3, 1)],
                    5: [lambda: qp_unit(3, 0)],
                    6: [lambda: qp_unit(0, 1)]},
                3: {1: [lambda: qp_unit(1, 1)],
                    5: [lambda: qp_unit(2, 1)]},
                4: {1: [lambda: qp_unit(3, 1)]},
                5: {5: [lambda: op_unit(0)], 6: [lambda: op_unit(1)]},
                6: {5: [lambda: op_unit(2)], 6: [lambda: op_unit(3)]},
                7: {5: [lambda: opk01_unit(4)], 6: [lambda: opk01_unit(5)]},
            }

            # ---- pre-loop + attention steps (c-outer) -------------------
            kp_unit(0, 0)
            qp_unit(0, 0)

            steps = [(j, c) for c in range(NCH) for j in range(HEADS // 2)]
            prev = None          # pv state awaiting its slot matmuls
            pending_norm = None  # broadcast + normalize of PV two steps back
            for i, (j, c) in enumerate(steps):
                e_gs = []
                cur = new_pv(j, c, e_gs, parity=i % 2)
                for t in range(NKT):
                    e_gs.append(s_group(j, c, t))
                    if prev is not None:
                        if t <= 3:
                            pv_emit(prev, 0, (2 * t, 2 * t + 1))
                        elif t == 5:
                            pv_emit(prev, 1, (0, 1, 2))
                        elif t == 6:
                            pv_emit(prev, 1, (3, 4, 5))
                        elif t == 7:
                            pv_emit(prev, 1, (6, 7))
                    if i == len(steps) - 1 and t >= 5:
                        # last pair's head0 PV interleaves into its own step
                        pv_emit(cur, 0, (2 * (t - 5), 2 * (t - 5) + 1))
                    if t == 4 and pending_norm is not None:
                        pending_norm()
                        pending_norm = None
                    for u in sched[i].get(t, []):
                        u()
                if prev is not None:
                    pending_norm = make_finish(prev)
                prev = cur

            # ---- tail: finish PV(3,1) + last two norms + split-k drain --
            pv_emit(prev, 0, (6, 7))       # + head0 den chop
            opk01_unit(6)
            opk01_unit(7)
            pending_norm()                 # norm of pair (2,1)
            pv_emit(prev, 1, tuple(range(NKT)))  # head1 + its den chain
            for m in range(4, NQT):
                opk2_unit(m)
            fin = make_finish(prev)
            fin()                          # norm of pair (3,1)
            for m in range(4, NQT):
                opk3_unit(m)

    nc.finalize()
    return nc


def _get_program():
    if "nc" not in _PROG_CACHE:
        _PROG_CACHE["nc"] = _build_program()
    return _PROG_CACHE["nc"]


def _consts():
    # mask for the 1/den partition broadcast: contraction row 64 carries the
    # even head's reciprocal (-> out partitions 0:64 where its O rows live),
    # row 0 carries the odd head's (-> out partitions 64:128)
    diag = np.zeros((128, 128), dtype=np.float32)
    diag[64, 0:64] = 1.0
    diag[0, 64:128] = 1.0
    zr = np.zeros((128, 512), dtype=np.float32)
    return diag, zr


def _prep_shared(Wq, Wk, Wv, Wo, bo):
    """Host-side weight tiling shared by all cores."""
    import ml_dtypes
    bf = ml_dtypes.bfloat16

    def _tile_w(w):  # [(ko p), n] -> [p, ko, n] contiguous bf16
        w = np.asarray(w, dtype=np.float32).astype(bf)
        ko = w.shape[0] // P
        return np.ascontiguousarray(
            w.reshape(ko, P, w.shape[1]).transpose(1, 0, 2)
        )

    def _tile_w_m(w):  # [(ko p), (m 128)] -> [m, p, ko, 128] contiguous
        w = np.asarray(w, dtype=np.float32).astype(bf)
        ko = w.shape[0] // P
        m = w.shape[1] // P
        return np.ascontiguousarray(
            w.reshape(ko, P, m, P).transpose(2, 1, 0, 3)
        )

    Wqb = _tile_w_m(Wq)
    Wkb = _tile_w_m(Wk)
    Wvb = _tile_w(Wv)
    Wob = _tile_w(Wo)
    bob = np.ascontiguousarray(np.asarray(bo, dtype=np.float32))
    return Wqb, Wkb, Wvb, Wob, bob


def _make_in_maps(x, context, Wq, Wk, Wv, Wo, bo):
    import ml_dtypes
    bf = ml_dtypes.bfloat16

    x = np.asarray(x, dtype=np.float32)
    context = np.asarray(context, dtype=np.float32)
    Wqb, Wkb, Wvb, Wob, bob = _prep_shared(Wq, Wk, Wv, Wo, bo)
    diag, zr = _consts()

    in_maps = []
    for core in range(N_CORES):
        b, half = divmod(core, 2)
        xs = np.ascontiguousarray(
            x[b, half * NQH:(half + 1) * NQH, :].T.astype(bf)
        )
        cs = np.ascontiguousarray(context[b].T.astype(bf))
        in_maps.append(
            {"xT": xs, "ctxT": cs, "Wq": Wqb, "Wk": Wkb, "Wv": Wvb,
             "Wo": Wob, "bo": bob, "diag2": diag, "zr": zr}
        )
    return in_maps


def kernel(x, context, Wq, Wk, Wv, Wo, bo, **_unused):
    from concourse.bass_utils import run_bass_kernel_spmd

    nc = _get_program()
    in_maps = _make_in_maps(x, context, Wq, Wk, Wv, Wo, bo)
    res = run_bass_kernel_spmd(nc, in_maps, core_ids=list(range(N_CORES)))

    out = np.empty((B, NQ, DQ), np.float32)
    for core in range(N_CORES):
        b, half = divmod(core, 2)
        out[b, half * NQH:(half + 1) * NQH, :] = (
            res.results[core]["Y"].astype(np.float32)
        )
    return out


# revision 11
# speedup vs baseline: 1.0587x; 1.0587x over previous
"""CrossAttention kernel for 8 Trainium2 NeuronCores.

Sharding: batch (4) x query-row-half (2) -> 8 shards, one per core. Each core
computes the full cross-attention for its 1024 query rows of one batch:
Q/K/V projections, 8 heads of attention, and the output projection. K/V
projections are recomputed by both cores sharing a batch (20% extra flops)
in exchange for zero collectives and a pure-SPMD single NEFF.

Layout trick: x and context are transposed (and cast to bf16) on the host so
the contraction dim lands on SBUF partitions with contiguous DMAs; all device
matmuls run without on-chip transposes:
  QT = Wq.T @ xT      (i on partitions)     KT = Wk.T @ ctxT
  V  = ctxT.T @ Wv    (natural [nk, i])
  ST_h = KT_h @ QT_h  ([nk, nq], K=64, head pairs in PE row groups)
  P = exp(ST * scale) (no max-subtraction; logits are ~N(0,1), safe range)
  O^T_h | den_h = [V_h | ones].T @ P  (denominator rides free in the M dim)
  Y = (O^T/den).T @ Wo + bo

Schedule (v4): engine queues execute in order, so the emission order IS the
schedule. Each of a step's 8 score groups is followed immediately by PV
matmuls of the previous step's pair (head0 2/slot in slots 0-3, head1
3/3/2 in slots 5-7) plus at most ~1 projection unit, so the ACT engine's
exp stream never waits more than a slot. The softmax denominator chain
(stage den row -> chop DMA -> reciprocal -> scatter DMA) launches per head
as soon as that head's accumulation stops; the 1/den broadcast + normalize
runs at slot 4 two steps later, which also bounds PV-accumulator lifetime
so 3 PSUM banks suffice (scores 4 + PV 3 + scratch 1 = 8). Input DMAs are
chunked in first-consumption order across the three queues (~100GB/s
each): scalar carries the small weight tiles and finishes before the
first exp, sync and gpsimd carry the bulk. V-pad ones blocks are memset
on gpsimd instead of DMAed. The final pair's head0 PV interleaves into
the last step and the last two normalizations overlap the split-k output
projection drain, so only ~10us trails the last exp. Y is stored bf16
(upcast on host) to halve the output DMA.
"""

import numpy as np

HEADS = 8
DIM_HEAD = 64
SCALE = DIM_HEAD ** -0.5
B, NQ, DQ = 4, 2048, 512
NK, DC = 1024, 768
INNER = HEADS * DIM_HEAD  # 512
NQH = NQ // 2             # query rows per core
N_CORES = 8
P = 128

_PROG_CACHE = {}


def _build_program():
    import concourse.bacc as bacc
    import concourse.tile as tile
    from concourse import mybir
    from concourse.bass import ts, ds

    f32 = mybir.dt.float32
    f32r = mybir.dt.float32r
    bf16 = mybir.dt.bfloat16
    Exp = mybir.ActivationFunctionType.Exp

    nc = bacc.Bacc(
        "TRN2",
        target_bir_lowering=False,
        debug=False,
        num_devices=N_CORES,
    )

    KQ = DQ // P      # 4  k-tiles for x-side contraction
    KC = DC // P      # 6  k-tiles for context-side contraction
    KI = INNER // P   # 4  k-tiles for inner-dim contraction
    NQT = NQH // P    # 8  query row tiles
    NKT = NK // P     # 8  key row tiles
    NCH = NQH // 512  # 2  nq chunks of 512

    # Wq/Wk host-tiled [m, p, ko, 128] so per-m loads are contiguous.
    xT_d = nc.dram_tensor("xT", [DQ, NQH], bf16, kind="ExternalInput")
    ctxT_d = nc.dram_tensor("ctxT", [DC, NK], bf16, kind="ExternalInput")
    Wq_d = nc.dram_tensor("Wq", [KQ, P, KQ, P], bf16, kind="ExternalInput")
    Wk_d = nc.dram_tensor("Wk", [KQ, P, KC, P], bf16, kind="ExternalInput")
    Wv_d = nc.dram_tensor("Wv", [P, KC, INNER], bf16, kind="ExternalInput")
    Wo_d = nc.dram_tensor("Wo", [P, KI, DQ], bf16, kind="ExternalInput")
    bo_d = nc.dram_tensor("bo", [DQ], f32, kind="ExternalInput")
    diag_d = nc.dram_tensor("diag2", [P, 128], bf16, kind="ExternalInput")
    Y_d = nc.dram_tensor("Y", [NQH, DQ], bf16, kind="ExternalOutput")

    with tile.TileContext(nc) as tc:
        with (
            tc.tile_pool(name="consts", bufs=1) as consts,
            tc.tile_pool(name="sc", bufs=2, space="PSUM") as scp,
            tc.tile_pool(name="pv", bufs=3, space="PSUM") as pvp,
            tc.tile_pool(name="mm", bufs=1, space="PSUM") as mmp,
            tc.tile_pool(name="ep", bufs=16) as ep,
            tc.tile_pool(name="dn", bufs=2) as dnp,
            tc.tile_pool(name="yp", bufs=2) as yp,
        ):
            Wk_sb = consts.tile([P, KQ, KC, P], bf16, tag="wk")
            Wq_sb = consts.tile([P, KQ, KQ, P], bf16, tag="wq")
            Wv_sb = consts.tile([P, KC, INNER], bf16, tag="wv")
            Wo_sb = consts.tile([P, KI, DQ], bf16, tag="wo")
            ctx_sb = consts.tile([P, KC, NK], bf16, tag="ctx")
            xT_sb = consts.tile([P, KQ, NQH], bf16, tag="x")
            bo_sb = consts.tile([P, DQ], f32, tag="bo")
            diag_sb = consts.tile([P, P], bf16, tag="diag")
            # two independent den-chain buffer sets alternating per step so
            # consecutive normalizations never serialize on WAR hazards.
            # r rows != {0, 64} must read as exact zeros for the mask-matmul
            # broadcast (0 * garbage-NaN would poison it): bf16 so memset
            # can clear them (and the broadcast matmul runs at bf16 rate).
            r_a = consts.tile([P, 512], bf16, tag="ra")
            r_b = consts.tile([P, 512], bf16, tag="rb2")
            d_a = consts.tile([P, 512], f32, tag="da")
            d_b = consts.tile([P, 512], f32, tag="db")

            ctx_src = ctxT_d.ap().rearrange("(ko p) n -> p ko n", p=P)
            xT_src = xT_d.ap().rearrange("(ko p) n -> p ko n", p=P)

            # ---- input DMA emission, in first-consumption order ----------
            # The three queues share the DMA engine pool (~60-100GB/s per
            # queue when all active); the first-needed chunks lead on every
            # queue. The issue (~0.7us engine time) is what occupies the
            # engine, transfers stream behind; scalar's issues all finish
            # before the first exp needs the ACT engine.
            nc.scalar.dma_start(out=Wk_sb[:, 0], in_=Wk_d.ap()[0])
            nc.sync.dma_start(
                out=ctx_sb[:, 0:2, 0:512], in_=ctx_src[:, 0:2, 0:512]
            )
            nc.gpsimd.dma_start(
                out=xT_sb[:, :, 0:512], in_=xT_src[:, :, 0:512]
            )
            nc.scalar.dma_start(
                out=ctx_sb[:, 4:6, 0:512], in_=ctx_src[:, 4:6, 0:512]
            )
            nc.sync.dma_start(out=Wq_sb[:, 0], in_=Wq_d.ap()[0])
            nc.gpsimd.dma_start(
                out=ctx_sb[:, 2:4, 0:512], in_=ctx_src[:, 2:4, 0:512]
            )
            nc.scalar.dma_start(out=Wq_sb[:, 1], in_=Wq_d.ap()[1])
            nc.sync.dma_start(out=Wk_sb[:, 1], in_=Wk_d.ap()[1])
            nc.gpsimd.dma_start(
                out=ctx_sb[:, 0:3, 512:1024], in_=ctx_src[:, 0:3, 512:1024]
            )
            nc.scalar.dma_start(out=Wk_sb[:, 2], in_=Wk_d.ap()[2])
            nc.sync.dma_start(
                out=Wv_sb[:, 0:3, :], in_=Wv_d.ap()[:, 0:3, :]
            )
            nc.gpsimd.dma_start(
                out=ctx_sb[:, 3:6, 512:1024], in_=ctx_src[:, 3:6, 512:1024]
            )
            nc.scalar.dma_start(out=Wq_sb[:, 2], in_=Wq_d.ap()[2])
            nc.sync.dma_start(
                out=Wv_sb[:, 3:6, :], in_=Wv_d.ap()[:, 3:6, :]
            )
            nc.scalar.dma_start(out=Wk_sb[:, 3], in_=Wk_d.ap()[3])
            nc.scalar.dma_start(out=Wq_sb[:, 3], in_=Wq_d.ap()[3])
            nc.gpsimd.dma_start(out=diag_sb, in_=diag_d.ap())
            nc.sync.dma_start(
                out=xT_sb[:, :, 512:1024], in_=xT_src[:, :, 512:1024]
            )
            nc.sync.dma_start(out=Wo_sb, in_=Wo_d.ap())
            nc.gpsimd.dma_start(
                out=bo_sb, in_=bo_d.ap().unsqueeze(0).to_broadcast((P, DQ))
            )
            nc.gpsimd.memset(r_a, 0.0)
            nc.gpsimd.memset(r_b, 0.0)

            KT_sb = consts.tile([P, KI, NK], bf16, tag="kt")    # [i, nk]
            QT_sb = consts.tile([P, KI, NQH], bf16, tag="qt")   # [i, nq]
            # V in natural [nk, i] layout padded per head to 128 cols:
            # even head h: cols h*128+[0:64]=V_h, [64:128]=ones
            # odd  head h: cols h*128+[0:64]=ones, [64:128]=V_h
            V_sb = consts.tile([P, NKT, HEADS * P], bf16, tag="v")
            OT_sb = consts.tile([P, KI, NQH], bf16, tag="ot")   # [i, nq]

            for t in range(NKT):
                dv4 = V_sb[:, t, :].rearrange("p (j y) -> p j y", j=4)
                nc.gpsimd.memset(dv4[:, :, 64:192], 1.0)

            # ---- PE work units (emitted as schedule filler) ----
            def kp_unit(m, c):  # K projection: KT[:, m, c*512:...]
                psk = mmp.tile([P, 512], f32, tag="acc")
                for k in range(KC):
                    nc.tensor.matmul(
                        psk,
                        lhsT=Wk_sb[:, m, k, :],
                        rhs=ctx_sb[:, k, ds(c * 512, 512)],
                        start=(k == 0),
                        stop=(k == KC - 1),
                    )
                nc.vector.tensor_copy(KT_sb[:, m, ds(c * 512, 512)], psk)

            def qp_unit(m, c):  # Q projection: QT[:, m, c*512:...]
                psq = mmp.tile([P, 512], f32, tag="acc")
                for k in range(KQ):
                    nc.tensor.matmul(
                        psq,
                        lhsT=Wq_sb[:, m, k, :],
                        rhs=xT_sb[:, k, ds(c * 512, 512)],
                        start=(k == 0),
                        stop=(k == KQ - 1),
                    )
                nc.vector.tensor_copy(QT_sb[:, m, ds(c * 512, 512)], psq)

            def vp_unit(t):  # V projection tile t, scattered into head pads
                psv = mmp.tile([P, 512], f32, tag="acc")
                for k in range(KC):
                    nc.tensor.matmul(
                        psv,
                        lhsT=ctx_sb[:, k, ts(t, P)],
                        rhs=Wv_sb[:, k, :],
                        start=(k == 0),
                        stop=(k == KC - 1),
                    )
                pv4 = psv.rearrange("p (j x) -> p j x", j=4)
                dv4 = V_sb[:, t, :].rearrange("p (j y) -> p j y", j=4)
                nc.vector.tensor_copy(dv4[:, :, 0:64], pv4[:, :, 0:64])
                nc.vector.tensor_copy(dv4[:, :, 192:256], pv4[:, :, 64:128])

            def op_unit(m):  # output projection row tile m (full k)
                psy = mmp.tile([P, 512], f32, tag="acc")
                for k in range(KI):
                    nc.tensor.matmul(
                        psy,
                        lhsT=OT_sb[:, k, ts(m, P)],
                        rhs=Wo_sb[:, k, :],
                        start=(k == 0),
                        stop=(k == KI - 1),
                    )
                y_t = yp.tile([P, DQ], bf16, tag="y")
                nc.vector.tensor_tensor(y_t, psy, bo_sb, op=mybir.AluOpType.add)
                nc.sync.dma_start(out=Y_d.ap()[ts(m, P), :], in_=y_t)

            # split-k output projection for the last 4 row tiles: k=0..1 run
            # during the final step, k=2/k=3 drain between the last two
            # normalizations at the tail
            partials = {}

            def opk01_unit(m):
                psy = mmp.tile([P, 512], f32, tag="acc")
                for k in range(2):
                    nc.tensor.matmul(
                        psy,
                        lhsT=OT_sb[:, k, ts(m, P)],
                        rhs=Wo_sb[:, k, :],
                        start=(k == 0),
                        stop=(k == 1),
                    )
                part = yp.tile([P, 512], f32, tag="part", bufs=4)
                nc.vector.tensor_tensor(
                    part, psy, bo_sb, op=mybir.AluOpType.add
                )
                partials[m] = part

            def opk2_unit(m):
                psy = mmp.tile([P, 512], f32, tag="acc")
                nc.tensor.matmul(
                    psy, lhsT=OT_sb[:, 2, ts(m, P)], rhs=Wo_sb[:, 2, :],
                    start=True, stop=True,
                )
                part2 = yp.tile([P, 512], f32, tag="part2", bufs=4)
                nc.vector.tensor_tensor(
                    part2, psy, partials[m], op=mybir.AluOpType.add
                )
                partials[m] = part2

            def opk3_unit(m):
                psy = mmp.tile([P, 512], f32, tag="acc")
                nc.tensor.matmul(
                    psy, lhsT=OT_sb[:, 3, ts(m, P)], rhs=Wo_sb[:, 3, :],
                    start=True, stop=True,
                )
                y_t = yp.tile([P, DQ], bf16, tag="y")
                nc.vector.tensor_tensor(
                    y_t, psy, partials[m], op=mybir.AluOpType.add
                )
                nc.sync.dma_start(out=Y_d.ap()[ts(m, P), :], in_=y_t)

            def s_group(j, c, t):  # one nk-tile of scores for head pair j
                ps_g = scp.tile([P, 2, 512], f32, tag="s")
                e_g = ep.tile([P, 2, 512], bf16, tag="e")
                nc.tensor.matmul(
                    ps_g[:, 0, :],
                    lhsT=KT_sb[0:64, j, ts(t, P)],
                    rhs=QT_sb[0:64, j, ds(c * 512, 512)],
                    start=True, stop=True,
                )
                nc.tensor.matmul(
                    ps_g[:, 1, :],
                    lhsT=KT_sb[64:128, j, ts(t, P)],
                    rhs=QT_sb[64:128, j, ds(c * 512, 512)],
                    start=True, stop=True,
                )
                nc.scalar.activation(out=e_g, in_=ps_g, func=Exp, scale=SCALE)
                return e_g

            # ---- PV of a pair, emitted in per-slot chunks ---------------
            def new_pv(j, c, e_gs, parity):
                return {"j": j, "c": c, "e": e_gs, "par": parity,
                        "po": [None, None], "dr": None}

            def pv_emit(st, ab, tts):
                """PV matmuls for head `2j+ab` over nk tiles tts; when the
                accumulation stops (tt==7), stage that head's den row and
                launch its half of the reciprocal chain."""
                d_sb = d_a if st["par"] == 0 else d_b
                r_sb = r_a if st["par"] == 0 else r_b
                h = 2 * st["j"] + ab
                if st["po"][ab] is None:
                    st["po"][ab] = pvp.tile(
                        [P, 512], f32, tag="po", name=f"po{ab}"
                    )
                po = st["po"][ab]
                for tt in tts:
                    nc.tensor.matmul(
                        po,
                        lhsT=V_sb[:, tt, ds(h * P, P)],
                        rhs=st["e"][tt][:, ab, :],
                        start=(tt == 0),
                        stop=(tt == NKT - 1),
                    )
                if tts[-1] == NKT - 1:
                    if ab == 0:
                        nc.vector.tensor_copy(d_sb[64:65, :], po[64:65, :])
                        st["dr"] = dnp.tile(
                            [64, 16], f32, tag="dr", name="dr"
                        )
                        nc.gpsimd.dma_start(
                            out=st["dr"][:, 0:8], in_=d_sb[64:65, :]
                        )
                    else:
                        dr = st["dr"]
                        nc.vector.tensor_copy(d_sb[0:1, :], po[0:1, :])
                        nc.gpsimd.dma_start(
                            out=dr[:, 8:16], in_=d_sb[0:1, :]
                        )
                        rr = dnp.tile([64, 16], bf16, tag="rr")
                        with nc.allow_low_precision(
                            reason="1/den feeds an f32r matmul"
                        ):
                            nc.vector.reciprocal(rr, dr)
                        nc.gpsimd.dma_start(
                            out=r_sb[64:65, :], in_=rr[:, 0:8]
                        )
                        nc.gpsimd.dma_start(
                            out=r_sb[0:1, :], in_=rr[:, 8:16]
                        )

            def make_finish(st):
                r_sb = r_a if st["par"] == 0 else r_b
                j, c, pos = st["j"], st["c"], st["po"]

                def finish():
                    ps_rb = mmp.tile([P, 512], f32, tag="acc")
                    nc.tensor.matmul(
                        ps_rb, lhsT=diag_sb, rhs=r_sb, start=True, stop=True
                    )
                    # DVE may read only one PSUM operand per instruction
                    rb_sb = yp.tile([P, 512], f32, tag="rb")
                    nc.vector.tensor_copy(rb_sb, ps_rb)
                    csl = ds(c * 512, 512)
                    nc.vector.tensor_tensor(
                        OT_sb[0:64, j, csl], pos[0][0:64, :], rb_sb[0:64, :],
                        op=mybir.AluOpType.mult,
                    )
                    nc.vector.tensor_tensor(
                        OT_sb[64:128, j, csl], pos[1][64:128, :],
                        rb_sb[64:128, :], op=mybir.AluOpType.mult,
                    )
                return finish

            # per-step slot fillers: {step: {slot: [unit, ...]}}
            sched = {
                0: {2: [lambda: vp_unit(0)],
                    3: [lambda: kp_unit(0, 1)],
                    4: [lambda: vp_unit(1)],
                    5: [lambda: kp_unit(1, 0)],
                    6: [lambda: qp_unit(1, 0)],
                    7: [lambda: kp_unit(1, 1), lambda: vp_unit(2)]},
                1: {0: [lambda: vp_unit(3), lambda: vp_unit(4)],
                    1: [lambda: vp_unit(5)],
                    2: [lambda: vp_unit(6), lambda: vp_unit(7)],
                    5: [lambda: kp_unit(2, 0)],
                    6: [lambda: kp_unit(2, 1)],
                    7: [lambda: qp_unit(2, 0)]},
                2: {1: [lambda: kp_unit(3, 0)],
                    3: [lambda: kp_unit(3, 1)],
                    5: [lambda: qp_unit(3, 0)],
                    6: [lambda: qp_unit(0, 1)]},
                3: {1: [lambda: qp_unit(1, 1)],
                    5: [lambda: qp_unit(2, 1)]},
                4: {1: [lambda: qp_unit(3, 1)]},
                5: {5: [lambda: op_unit(0)], 6: [lambda: op_unit(1)]},
                6: {5: [lambda: op_unit(2)], 6: [lambda: op_unit(3)]},
                7: {5: [lambda: opk01_unit(4)], 6: [lambda: opk01_unit(5)]},
            }

            # ---- pre-loop + attention steps (c-outer) -------------------
            kp_unit(0, 0)
            qp_unit(0, 0)

            steps = [(j, c) for c in range(NCH) for j in range(HEADS // 2)]
            prev = None          # pv state awaiting its slot matmuls
            pending_norm = None  # broadcast + normalize of PV two steps back
            for i, (j, c) in enumerate(steps):
                e_gs = []
                cur = new_pv(j, c, e_gs, parity=i % 2)
                for t in range(NKT):
                    e_gs.append(s_group(j, c, t))
                    if prev is not None:
                        if t <= 3:
                            pv_emit(prev, 0, (2 * t, 2 * t + 1))
                        elif t == 5:
                            pv_emit(prev, 1, (0, 1, 2))
                        elif t == 6:
                            pv_emit(prev, 1, (3, 4, 5))
                        elif t == 7:
                            pv_emit(prev, 1, (6, 7))
                    if i == len(steps) - 1 and t >= 5:
                        # last pair's head0 PV interleaves into its own step
                        pv_emit(cur, 0, (2 * (t - 5), 2 * (t - 5) + 1))
                    if t == 4 and pending_norm is not None:
                        pending_norm()
                        pending_norm = None
                    for u in sched[i].get(t, []):
                        u()
                if prev is not None:
                    pending_norm = make_finish(prev)
                prev = cur

            # ---- tail: finish PV(3,1) + last two norms + split-k drain --
            pv_emit(prev, 0, (6, 7))       # + head0 den chop
            opk01_unit(6)
            opk01_unit(7)
            pending_norm()                 # norm of pair (2,1)
            pv_emit(prev, 1, tuple(range(NKT)))  # head1 + its den chain
            for m in range(4, NQT):
                opk2_unit(m)
            fin = make_finish(prev)
            fin()                          # norm of pair (3,1)
            for m in range(4, NQT):
                opk3_unit(m)

    nc.finalize()
    return nc


def _get_program():
    if "nc" not in _PROG_CACHE:
        _PROG_CACHE["nc"] = _build_program()
    return _PROG_CACHE["nc"]


def _consts():
    import ml_dtypes
    # mask for the 1/den partition broadcast: contraction row 64 carries the
    # even head's reciprocal (-> out partitions 0:64 where its O rows live),
    # row 0 carries the odd head's (-> out partitions 64:128)
    diag = np.zeros((128, 128), dtype=ml_dtypes.bfloat16)
    diag[64, 0:64] = 1.0
    diag[0, 64:128] = 1.0
    return diag


def _prep_shared(Wq, Wk, Wv, Wo, bo):
    """Host-side weight tiling shared by all cores."""
    import ml_dtypes
    bf = ml_dtypes.bfloat16

    def _tile_w(w):  # [(ko p), n] -> [p, ko, n] contiguous bf16
        w = np.asarray(w, dtype=np.float32).astype(bf)
        ko = w.shape[0] // P
        return np.ascontiguousarray(
            w.reshape(ko, P, w.shape[1]).transpose(1, 0, 2)
        )

    def _tile_w_m(w):  # [(ko p), (m 128)] -> [m, p, ko, 128] contiguous
        w = np.asarray(w, dtype=np.float32).astype(bf)
        ko = w.shape[0] // P
        m = w.shape[1] // P
        return np.ascontiguousarray(
            w.reshape(ko, P, m, P).transpose(2, 1, 0, 3)
        )

    Wqb = _tile_w_m(Wq)
    Wkb = _tile_w_m(Wk)
    Wvb = _tile_w(Wv)
    Wob = _tile_w(Wo)
    bob = np.ascontiguousarray(np.asarray(bo, dtype=np.float32))
    return Wqb, Wkb, Wvb, Wob, bob


def _make_in_maps(x, context, Wq, Wk, Wv, Wo, bo):
    import ml_dtypes
    bf = ml_dtypes.bfloat16

    x = np.asarray(x, dtype=np.float32)
    context = np.asarray(context, dtype=np.float32)
    Wqb, Wkb, Wvb, Wob, bob = _prep_shared(Wq, Wk, Wv, Wo, bo)
    diag = _consts()

    in_maps = []
    for core in range(N_CORES):
        b, half = divmod(core, 2)
        xs = np.ascontiguousarray(
            x[b, half * NQH:(half + 1) * NQH, :].T.astype(bf)
        )
        cs = np.ascontiguousarray(context[b].T.astype(bf))
        in_maps.append(
            {"xT": xs, "ctxT": cs, "Wq": Wqb, "Wk": Wkb, "Wv": Wvb,
             "Wo": Wob, "bo": bob, "diag2": diag}
        )
    return in_maps


def kernel(x, context, Wq, Wk, Wv, Wo, bo, **_unused):
    from concourse.bass_utils import run_bass_kernel_spmd

    nc = _get_program()
    in_maps = _make_in_maps(x, context, Wq, Wk, Wv, Wo, bo)
    res = run_bass_kernel_spmd(nc, in_maps, core_ids=list(range(N_CORES)))

    out = np.empty((B, NQ, DQ), np.float32)
    for core in range(N_CORES):
        b, half = divmod(core, 2)
        out[b, half * NQH:(half + 1) * NQH, :] = (
            res.results[core]["Y"].astype(np.float32)
        )
    return out


# revision 14
# speedup vs baseline: 1.0897x; 1.0293x over previous
"""CrossAttention kernel for 8 Trainium2 NeuronCores.

Sharding: batch (4) x query-row-half (2) -> 8 shards, one per core. Each core
computes the full cross-attention for its 1024 query rows of one batch:
Q/K/V projections, 8 heads of attention, and the output projection. K/V
projections are recomputed by both cores sharing a batch (20% extra flops)
in exchange for zero collectives and a pure-SPMD single NEFF.

Layout trick: x and context are transposed (and cast to bf16) on the host so
the contraction dim lands on SBUF partitions with contiguous DMAs; all device
matmuls run without on-chip transposes:
  QT = Wq.T @ xT      (i on partitions)     KT = Wk.T @ ctxT
  V  = ctxT.T @ Wv    (natural [nk, i])
  ST_h = KT_h @ QT_h  ([nk, nq], K=64, head pairs in PE row groups)
  P = exp(ST * scale) (no max-subtraction; logits are ~N(0,1), safe range)
  O^T_h | den_h = [V_h | ones].T @ P  (denominator rides free in the M dim)
  Y = (O^T/den).T @ Wo + bo

Schedule (v4): engine queues execute in order, so the emission order IS the
schedule. Each of a step's 8 score groups is followed immediately by PV
matmuls of the previous step's pair (head0 2/slot in slots 0-3, head1
3/3/2 in slots 5-7) plus at most ~1 projection unit, so the ACT engine's
exp stream never waits more than a slot. The softmax denominator chain
(stage den row -> chop DMA -> reciprocal -> scatter DMA) launches per head
as soon as that head's accumulation stops; the 1/den broadcast + normalize
runs at slot 4 two steps later, which also bounds PV-accumulator lifetime
so 3 PSUM banks suffice (scores 4 + PV 3 + scratch 1 = 8). Input DMAs are
chunked in first-consumption order across the three queues (~100GB/s
each): scalar carries the small weight tiles and finishes before the
first exp, sync and gpsimd carry the bulk. V-pad ones blocks are memset
on gpsimd instead of DMAed. The final pair's head0 PV interleaves into
the last step and the last two normalizations overlap the split-k output
projection drain, so only ~10us trails the last exp. Y is stored bf16
(upcast on host) to halve the output DMA.
"""

import numpy as np

HEADS = 8
DIM_HEAD = 64
SCALE = DIM_HEAD ** -0.5
B, NQ, DQ = 4, 2048, 512
NK, DC = 1024, 768
INNER = HEADS * DIM_HEAD  # 512
NQH = NQ // 2             # query rows per core
N_CORES = 8
P = 128

_PROG_CACHE = {}


def _build_program():
    import concourse.bacc as bacc
    import concourse.tile as tile
    from concourse import mybir
    from concourse.bass import ts, ds

    f32 = mybir.dt.float32
    f32r = mybir.dt.float32r
    bf16 = mybir.dt.bfloat16
    Exp = mybir.ActivationFunctionType.Exp

    nc = bacc.Bacc(
        "TRN2",
        target_bir_lowering=False,
        debug=False,
        num_devices=N_CORES,
    )

    KQ = DQ // P      # 4  k-tiles for x-side contraction
    KC = DC // P      # 6  k-tiles for context-side contraction
    KI = INNER // P   # 4  k-tiles for inner-dim contraction
    NQT = NQH // P    # 8  query row tiles
    NKT = NK // P     # 8  key row tiles
    NCH = NQH // 512  # 2  nq chunks of 512

    # Wq/Wk host-tiled [m, p, ko, 128] so per-m loads are contiguous.
    xT_d = nc.dram_tensor("xT", [DQ, NQH], bf16, kind="ExternalInput")
    ctxT_d = nc.dram_tensor("ctxT", [DC, NK], bf16, kind="ExternalInput")
    Wq_d = nc.dram_tensor("Wq", [KQ, P, KQ, P], bf16, kind="ExternalInput")
    Wk_d = nc.dram_tensor("Wk", [KQ, P, KC, P], bf16, kind="ExternalInput")
    Wv_d = nc.dram_tensor("Wv", [P, KC, INNER], bf16, kind="ExternalInput")
    Wo_d = nc.dram_tensor("Wo", [P, KI, DQ], bf16, kind="ExternalInput")
    bo_d = nc.dram_tensor("bo", [DQ], f32, kind="ExternalInput")
    diag_d = nc.dram_tensor("diag2", [P, 128], bf16, kind="ExternalInput")
    Y_d = nc.dram_tensor("Y", [NQH, DQ], bf16, kind="ExternalOutput")

    with tile.TileContext(nc) as tc:
        with (
            tc.tile_pool(name="consts", bufs=1) as consts,
            tc.tile_pool(name="sc", bufs=2, space="PSUM") as scp,
            tc.tile_pool(name="pv", bufs=3, space="PSUM") as pvp,
            tc.tile_pool(name="mm", bufs=1, space="PSUM") as mmp,
            tc.tile_pool(name="ep", bufs=16) as ep,
            tc.tile_pool(name="dn", bufs=2) as dnp,
            tc.tile_pool(name="yp", bufs=2) as yp,
        ):
            Wk_sb = consts.tile([P, KQ, KC, P], bf16, tag="wk")
            Wq_sb = consts.tile([P, KQ, KQ, P], bf16, tag="wq")
            Wv_sb = consts.tile([P, KC, INNER], bf16, tag="wv")
            Wo_sb = consts.tile([P, KI, DQ], bf16, tag="wo")
            ctx_sb = consts.tile([P, KC, NK], bf16, tag="ctx")
            xT_sb = consts.tile([P, KQ, NQH], bf16, tag="x")
            bo_sb = consts.tile([P, DQ], f32, tag="bo")
            diag_sb = consts.tile([P, P], bf16, tag="diag")
            # two independent den-chain buffer sets alternating per step so
            # consecutive normalizations never serialize on WAR hazards.
            # r rows != {0, 64} must read as exact zeros for the mask-matmul
            # broadcast (0 * garbage-NaN would poison it): bf16 so memset
            # can clear them (and the broadcast matmul runs at bf16 rate).
            r_a = consts.tile([P, 512], bf16, tag="ra")
            r_b = consts.tile([P, 512], bf16, tag="rb2")
            d_a = consts.tile([P, 512], f32, tag="da")
            d_b = consts.tile([P, 512], f32, tag="db")

            ctx_src = ctxT_d.ap().rearrange("(ko p) n -> p ko n", p=P)
            xT_src = xT_d.ap().rearrange("(ko p) n -> p ko n", p=P)

            # ---- input DMA emission, in first-consumption order ----------
            # The three queues share the DMA engine pool (~60-100GB/s per
            # queue when all active); the first-needed chunks lead on every
            # queue. The issue (~0.7us engine time) is what occupies the
            # engine, transfers stream behind; scalar's issues all finish
            # before the first exp needs the ACT engine.
            nc.scalar.dma_start(out=Wk_sb[:, 0], in_=Wk_d.ap()[0])
            nc.sync.dma_start(
                out=ctx_sb[:, 0:1, 0:512], in_=ctx_src[:, 0:1, 0:512]
            )
            nc.gpsimd.dma_start(
                out=xT_sb[:, 0:2, 0:512], in_=xT_src[:, 0:2, 0:512]
            )
            nc.scalar.dma_start(
                out=ctx_sb[:, 4:5, 0:512], in_=ctx_src[:, 4:5, 0:512]
            )
            nc.sync.dma_start(out=Wq_sb[:, 0], in_=Wq_d.ap()[0])
            nc.gpsimd.dma_start(
                out=ctx_sb[:, 2:3, 0:512], in_=ctx_src[:, 2:3, 0:512]
            )
            nc.scalar.dma_start(
                out=ctx_sb[:, 5:6, 0:512], in_=ctx_src[:, 5:6, 0:512]
            )
            nc.sync.dma_start(
                out=ctx_sb[:, 1:2, 0:512], in_=ctx_src[:, 1:2, 0:512]
            )
            nc.gpsimd.dma_start(
                out=xT_sb[:, 2:4, 0:512], in_=xT_src[:, 2:4, 0:512]
            )
            nc.scalar.dma_start(out=Wq_sb[:, 1], in_=Wq_d.ap()[1])
            nc.sync.dma_start(
                out=ctx_sb[:, 3:6, 512:1024], in_=ctx_src[:, 3:6, 512:1024]
            )
            nc.gpsimd.dma_start(
                out=ctx_sb[:, 3:4, 0:512], in_=ctx_src[:, 3:4, 0:512]
            )
            nc.scalar.dma_start(out=Wk_sb[:, 2], in_=Wk_d.ap()[2])
            nc.sync.dma_start(out=Wk_sb[:, 1], in_=Wk_d.ap()[1])
            nc.gpsimd.dma_start(
                out=ctx_sb[:, 0:3, 512:1024], in_=ctx_src[:, 0:3, 512:1024]
            )
            nc.scalar.dma_start(out=Wq_sb[:, 2], in_=Wq_d.ap()[2])
            nc.sync.dma_start(
                out=Wv_sb[:, 0:3, :], in_=Wv_d.ap()[:, 0:3, :]
            )
            nc.scalar.dma_start(out=Wk_sb[:, 3], in_=Wk_d.ap()[3])
            nc.sync.dma_start(
                out=Wv_sb[:, 3:6, :], in_=Wv_d.ap()[:, 3:6, :]
            )
            nc.scalar.dma_start(out=Wq_sb[:, 3], in_=Wq_d.ap()[3])
            nc.gpsimd.dma_start(out=diag_sb, in_=diag_d.ap())
            nc.sync.dma_start(
                out=xT_sb[:, :, 512:1024], in_=xT_src[:, :, 512:1024]
            )
            nc.sync.dma_start(out=Wo_sb, in_=Wo_d.ap())
            nc.gpsimd.dma_start(
                out=bo_sb, in_=bo_d.ap().unsqueeze(0).to_broadcast((P, DQ))
            )
            nc.gpsimd.memset(r_a, 0.0)
            nc.gpsimd.memset(r_b, 0.0)

            KT_sb = consts.tile([P, KI, NK], bf16, tag="kt")    # [i, nk]
            QT_sb = consts.tile([P, KI, NQH], bf16, tag="qt")   # [i, nq]
            # V in natural [nk, i] layout padded per head to 128 cols:
            # even head h: cols h*128+[0:64]=V_h, [64:128]=ones
            # odd  head h: cols h*128+[0:64]=ones, [64:128]=V_h
            V_sb = consts.tile([P, NKT, HEADS * P], bf16, tag="v")
            OT_sb = consts.tile([P, KI, NQH], bf16, tag="ot")   # [i, nq]

            for t in range(NKT):
                dv4 = V_sb[:, t, :].rearrange("p (j y) -> p j y", j=4)
                nc.gpsimd.memset(dv4[:, :, 64:192], 1.0)

            # ---- PE work units (emitted as schedule filler) ----
            def kp_unit(m, c):  # K projection: KT[:, m, c*512:...]
                psk = mmp.tile([P, 512], f32, tag="acc")
                for k in range(KC):
                    nc.tensor.matmul(
                        psk,
                        lhsT=Wk_sb[:, m, k, :],
                        rhs=ctx_sb[:, k, ds(c * 512, 512)],
                        start=(k == 0),
                        stop=(k == KC - 1),
                    )
                nc.vector.tensor_copy(KT_sb[:, m, ds(c * 512, 512)], psk)

            def qp_unit(m, c):  # Q projection: QT[:, m, c*512:...]
                psq = mmp.tile([P, 512], f32, tag="acc")
                for k in range(KQ):
                    nc.tensor.matmul(
                        psq,
                        lhsT=Wq_sb[:, m, k, :],
                        rhs=xT_sb[:, k, ds(c * 512, 512)],
                        start=(k == 0),
                        stop=(k == KQ - 1),
                    )
                nc.vector.tensor_copy(QT_sb[:, m, ds(c * 512, 512)], psq)

            def vp_unit(t):  # V projection tile t, scattered into head pads
                psv = mmp.tile([P, 512], f32, tag="acc")
                for k in range(KC):
                    nc.tensor.matmul(
                        psv,
                        lhsT=ctx_sb[:, k, ts(t, P)],
                        rhs=Wv_sb[:, k, :],
                        start=(k == 0),
                        stop=(k == KC - 1),
                    )
                pv4 = psv.rearrange("p (j x) -> p j x", j=4)
                dv4 = V_sb[:, t, :].rearrange("p (j y) -> p j y", j=4)
                nc.vector.tensor_copy(dv4[:, :, 0:64], pv4[:, :, 0:64])
                nc.vector.tensor_copy(dv4[:, :, 192:256], pv4[:, :, 64:128])

            def op_unit(m):  # output projection row tile m (full k)
                psy = mmp.tile([P, 512], f32, tag="acc")
                for k in range(KI):
                    nc.tensor.matmul(
                        psy,
                        lhsT=OT_sb[:, k, ts(m, P)],
                        rhs=Wo_sb[:, k, :],
                        start=(k == 0),
                        stop=(k == KI - 1),
                    )
                y_t = yp.tile([P, DQ], bf16, tag="y")
                nc.vector.tensor_tensor(y_t, psy, bo_sb, op=mybir.AluOpType.add)
                nc.sync.dma_start(out=Y_d.ap()[ts(m, P), :], in_=y_t)

            # split-k output projection for the last 4 row tiles: k=0..1 run
            # during the final step, k=2/k=3 drain between the last two
            # normalizations at the tail
            partials = {}

            def opk01_unit(m):
                psy = mmp.tile([P, 512], f32, tag="acc")
                for k in range(2):
                    nc.tensor.matmul(
                        psy,
                        lhsT=OT_sb[:, k, ts(m, P)],
                        rhs=Wo_sb[:, k, :],
                        start=(k == 0),
                        stop=(k == 1),
                    )
                part = yp.tile([P, 512], f32, tag="part", bufs=4)
                nc.vector.tensor_tensor(
                    part, psy, bo_sb, op=mybir.AluOpType.add
                )
                partials[m] = part

            def opk2_unit(m):
                psy = mmp.tile([P, 512], f32, tag="acc")
                nc.tensor.matmul(
                    psy, lhsT=OT_sb[:, 2, ts(m, P)], rhs=Wo_sb[:, 2, :],
                    start=True, stop=True,
                )
                part2 = yp.tile([P, 512], f32, tag="part2", bufs=4)
                nc.vector.tensor_tensor(
                    part2, psy, partials[m], op=mybir.AluOpType.add
                )
                partials[m] = part2

            def opk3_unit(m):
                psy = mmp.tile([P, 512], f32, tag="acc")
                nc.tensor.matmul(
                    psy, lhsT=OT_sb[:, 3, ts(m, P)], rhs=Wo_sb[:, 3, :],
                    start=True, stop=True,
                )
                y_t = yp.tile([P, DQ], bf16, tag="y")
                nc.vector.tensor_tensor(
                    y_t, psy, partials[m], op=mybir.AluOpType.add
                )
                nc.sync.dma_start(out=Y_d.ap()[ts(m, P), :], in_=y_t)

            def s_group(j, c, t):  # one nk-tile of scores for head pair j
                ps_g = scp.tile([P, 2, 512], f32, tag="s")
                e_g = ep.tile([P, 2, 512], bf16, tag="e")
                nc.tensor.matmul(
                    ps_g[:, 0, :],
                    lhsT=KT_sb[0:64, j, ts(t, P)],
                    rhs=QT_sb[0:64, j, ds(c * 512, 512)],
                    start=True, stop=True,
                )
                nc.tensor.matmul(
                    ps_g[:, 1, :],
                    lhsT=KT_sb[64:128, j, ts(t, P)],
                    rhs=QT_sb[64:128, j, ds(c * 512, 512)],
                    start=True, stop=True,
                )
                nc.scalar.activation(out=e_g, in_=ps_g, func=Exp, scale=SCALE)
                return e_g

            # ---- PV of a pair, emitted in per-slot chunks ---------------
            def new_pv(j, c, e_gs, parity):
                return {"j": j, "c": c, "e": e_gs, "par": parity,
                        "po": [None, None], "dr": None}

            def eng_copy(eng, out, in_):
                if hasattr(eng, "tensor_copy"):
                    eng.tensor_copy(out, in_)
                else:
                    eng.copy(out, in_)

            def pv_emit(st, ab, tts, cp=None, dq=None):
                """PV matmuls for head `2j+ab` over nk tiles tts; when the
                accumulation stops (tt==7), stage that head's den row and
                launch its half of the reciprocal chain. cp: engine for the
                PSUM->SBUF den-row copy (DVE default; ACT at the tail where
                it is idle). dq: DMA queue for chop/scatter (gpsimd default;
                sync at the tail)."""
                cp = cp or nc.vector
                dq = dq or nc.gpsimd
                d_sb = d_a if st["par"] == 0 else d_b
                r_sb = r_a if st["par"] == 0 else r_b
                h = 2 * st["j"] + ab
                if st["po"][ab] is None:
                    st["po"][ab] = pvp.tile(
                        [P, 512], f32, tag="po", name=f"po{ab}"
                    )
                po = st["po"][ab]
                for tt in tts:
                    nc.tensor.matmul(
                        po,
                        lhsT=V_sb[:, tt, ds(h * P, P)],
                        rhs=st["e"][tt][:, ab, :],
                        start=(tt == 0),
                        stop=(tt == NKT - 1),
                    )
                if tts[-1] == NKT - 1:
                    if ab == 0:
                        eng_copy(cp, d_sb[64:65, :], po[64:65, :])
                        st["dr"] = dnp.tile(
                            [64, 16], f32, tag="dr", name="dr"
                        )
                        dq.dma_start(
                            out=st["dr"][:, 0:8], in_=d_sb[64:65, :]
                        )
                    else:
                        dr = st["dr"]
                        eng_copy(cp, d_sb[0:1, :], po[0:1, :])
                        dq.dma_start(out=dr[:, 8:16], in_=d_sb[0:1, :])
                        rr = dnp.tile([64, 16], bf16, tag="rr")
                        with nc.allow_low_precision(
                            reason="1/den feeds a bf16 matmul"
                        ):
                            nc.vector.reciprocal(rr, dr)
                        dq.dma_start(out=r_sb[64:65, :], in_=rr[:, 0:8])
                        dq.dma_start(out=r_sb[0:1, :], in_=rr[:, 8:16])

            def make_finish(st, cp=None):
                cp = cp or nc.vector
                r_sb = r_a if st["par"] == 0 else r_b
                j, c, pos = st["j"], st["c"], st["po"]

                def finish():
                    ps_rb = mmp.tile([P, 512], f32, tag="acc")
                    nc.tensor.matmul(
                        ps_rb, lhsT=diag_sb, rhs=r_sb, start=True, stop=True
                    )
                    # DVE may read only one PSUM operand per instruction
                    rb_sb = yp.tile([P, 512], f32, tag="rb")
                    eng_copy(cp, rb_sb, ps_rb)
                    csl = ds(c * 512, 512)
                    nc.vector.tensor_tensor(
                        OT_sb[0:64, j, csl], pos[0][0:64, :], rb_sb[0:64, :],
                        op=mybir.AluOpType.mult,
                    )
                    nc.vector.tensor_tensor(
                        OT_sb[64:128, j, csl], pos[1][64:128, :],
                        rb_sb[64:128, :], op=mybir.AluOpType.mult,
                    )
                return finish

            # per-step slot fillers: {step: {slot: [unit, ...]}}
            sched = {
                0: {3: [lambda: kp_unit(0, 1)],
                    5: [lambda: kp_unit(1, 0)],
                    6: [lambda: qp_unit(1, 0), lambda: vp_unit(0)],
                    7: [lambda: kp_unit(1, 1), lambda: vp_unit(1),
                        lambda: vp_unit(2)]},
                1: {0: [lambda: vp_unit(3), lambda: vp_unit(4)],
                    1: [lambda: vp_unit(5)],
                    2: [lambda: vp_unit(6), lambda: vp_unit(7)],
                    5: [lambda: kp_unit(2, 0)],
                    6: [lambda: kp_unit(2, 1)],
                    7: [lambda: qp_unit(2, 0)]},
                2: {1: [lambda: kp_unit(3, 0)],
                    3: [lambda: kp_unit(3, 1)],
                    5: [lambda: qp_unit(3, 0)],
                    7: [lambda: qp_unit(0, 1)]},
                3: {1: [lambda: qp_unit(1, 1)],
                    7: [lambda: qp_unit(2, 1)]},
                4: {1: [lambda: qp_unit(3, 1)]},
                5: {5: [lambda: op_unit(0)], 7: [lambda: op_unit(1)]},
                6: {5: [lambda: op_unit(2)], 7: [lambda: op_unit(3)]},
                7: {7: [lambda: opk01_unit(4)]},
            }

            # ---- pre-loop + attention steps (c-outer) -------------------
            kp_unit(0, 0)
            qp_unit(0, 0)

            steps = [(j, c) for c in range(NCH) for j in range(HEADS // 2)]
            prev = None          # pv state awaiting its slot matmuls
            pending_norm = None  # broadcast + normalize of PV two steps back
            for i, (j, c) in enumerate(steps):
                e_gs = []
                cur = new_pv(j, c, e_gs, parity=i % 2)
                for t in range(NKT):
                    e_gs.append(s_group(j, c, t))
                    if prev is not None:
                        if t <= 3:
                            pv_emit(prev, 0, (2 * t, 2 * t + 1))
                        elif t == 5:
                            pv_emit(prev, 1, (0, 1, 2, 3))
                        elif t == 6:
                            pv_emit(prev, 1, (4, 5, 6, 7))
                    if i == len(steps) - 1 and t >= 5:
                        # last pair's head0 PV interleaves into its own step
                        pv_emit(cur, 0, (2 * (t - 5), 2 * (t - 5) + 1))
                    if t == 4 and pending_norm is not None:
                        pending_norm()
                        pending_norm = None
                    for u in sched[i].get(t, []):
                        u()
                if prev is not None:
                    pending_norm = make_finish(prev)
                prev = cur

            # ---- tail: finish PV(3,1) + last two norms + split-k drain --
            # the ACT engine is idle after the last exp: route the den-row
            # and 1/den-broadcast copies through it, and the den DMAs
            # through the otherwise-idle sync queue.
            pv_emit(prev, 0, (6, 7), cp=nc.scalar, dq=nc.sync)
            opk01_unit(5)
            opk01_unit(6)
            opk01_unit(7)
            pending_norm()                 # norm of pair (2,1)
            pv_emit(prev, 1, tuple(range(NKT)), cp=nc.scalar, dq=nc.sync)
            for m in range(4, NQT):
                opk2_unit(m)
            fin = make_finish(prev, cp=nc.scalar)
            fin()                          # norm of pair (3,1)
            for m in range(4, NQT):
                opk3_unit(m)

    nc.finalize()
    return nc


def _get_program():
    if "nc" not in _PROG_CACHE:
        _PROG_CACHE["nc"] = _build_program()
    return _PROG_CACHE["nc"]


def _consts():
    import ml_dtypes
    # mask for the 1/den partition broadcast: contraction row 64 carries the
    # even head's reciprocal (-> out partitions 0:64 where its O rows live),
    # row 0 carries the odd head's (-> out partitions 64:128)
    diag = np.zeros((128, 128), dtype=ml_dtypes.bfloat16)
    diag[64, 0:64] = 1.0
    diag[0, 64:128] = 1.0
    return diag


def _prep_shared(Wq, Wk, Wv, Wo, bo):
    """Host-side weight tiling shared by all cores."""
    import ml_dtypes
    bf = ml_dtypes.bfloat16

    def _tile_w(w):  # [(ko p), n] -> [p, ko, n] contiguous bf16
        w = np.asarray(w, dtype=np.float32).astype(bf)
        ko = w.shape[0] // P
        return np.ascontiguousarray(
            w.reshape(ko, P, w.shape[1]).transpose(1, 0, 2)
        )

    def _tile_w_m(w):  # [(ko p), (m 128)] -> [m, p, ko, 128] contiguous
        w = np.asarray(w, dtype=np.float32).astype(bf)
        ko = w.shape[0] // P
        m = w.shape[1] // P
        return np.ascontiguousarray(
            w.reshape(ko, P, m, P).transpose(2, 1, 0, 3)
        )

    Wqb = _tile_w_m(Wq)
    Wkb = _tile_w_m(Wk)
    Wvb = _tile_w(Wv)
    Wob = _tile_w(Wo)
    bob = np.ascontiguousarray(np.asarray(bo, dtype=np.float32))
    return Wqb, Wkb, Wvb, Wob, bob


def _make_in_maps(x, context, Wq, Wk, Wv, Wo, bo):
    import ml_dtypes
    bf = ml_dtypes.bfloat16

    x = np.asarray(x, dtype=np.float32)
    context = np.asarray(context, dtype=np.float32)
    Wqb, Wkb, Wvb, Wob, bob = _prep_shared(Wq, Wk, Wv, Wo, bo)
    diag = _consts()

    in_maps = []
    for core in range(N_CORES):
        b, half = divmod(core, 2)
        xs = np.ascontiguousarray(
            x[b, half * NQH:(half + 1) * NQH, :].T.astype(bf)
        )
        cs = np.ascontiguousarray(context[b].T.astype(bf))
        in_maps.append(
            {"xT": xs, "ctxT": cs, "Wq": Wqb, "Wk": Wkb, "Wv": Wvb,
             "Wo": Wob, "bo": bob, "diag2": diag}
        )
    return in_maps


def kernel(x, context, Wq, Wk, Wv, Wo, bo, **_unused):
    from concourse.bass_utils import run_bass_kernel_spmd

    nc = _get_program()
    in_maps = _make_in_maps(x, context, Wq, Wk, Wv, Wo, bo)
    res = run_bass_kernel_spmd(nc, in_maps, core_ids=list(range(N_CORES)))

    out = np.empty((B, NQ, DQ), np.float32)
    for core in range(N_CORES):
        b, half = divmod(core, 2)
        out[b, half * NQH:(half + 1) * NQH, :] = (
            res.results[core]["Y"].astype(np.float32)
        )
    return out


# revision 16
# speedup vs baseline: 1.0930x; 1.0030x over previous
"""CrossAttention kernel for 8 Trainium2 NeuronCores.

Sharding: batch (4) x query-row-half (2) -> 8 shards, one per core. Each core
computes the full cross-attention for its 1024 query rows of one batch:
Q/K/V projections, 8 heads of attention, and the output projection. K/V
projections are recomputed by both cores sharing a batch (20% extra flops)
in exchange for zero collectives and a pure-SPMD single NEFF.

Layout trick: x and context are transposed (and cast to bf16) on the host so
the contraction dim lands on SBUF partitions with contiguous DMAs; all device
matmuls run without on-chip transposes:
  QT = Wq.T @ xT      (i on partitions)     KT = Wk.T @ ctxT
  V  = ctxT.T @ Wv    (natural [nk, i])
  ST_h = KT_h @ QT_h  ([nk, nq], K=64, head pairs in PE row groups)
  P = exp(ST * scale) (no max-subtraction; logits are ~N(0,1), safe range)
  O^T_h | den_h = [V_h | ones].T @ P  (denominator rides free in the M dim)
  Y = (O^T/den).T @ Wo + bo

Schedule (v4): engine queues execute in order, so the emission order IS the
schedule. Each of a step's 8 score groups is followed immediately by PV
matmuls of the previous step's pair (head0 2/slot in slots 0-3, head1
3/3/2 in slots 5-7) plus at most ~1 projection unit, so the ACT engine's
exp stream never waits more than a slot. The softmax denominator chain
(stage den row -> chop DMA -> reciprocal -> scatter DMA) launches per head
as soon as that head's accumulation stops; the 1/den broadcast + normalize
runs at slot 4 two steps later, which also bounds PV-accumulator lifetime
so 3 PSUM banks suffice (scores 4 + PV 3 + scratch 1 = 8). Input DMAs are
chunked in first-consumption order across the three queues (~100GB/s
each): scalar carries the small weight tiles and finishes before the
first exp, sync and gpsimd carry the bulk. V-pad ones blocks are memset
on gpsimd instead of DMAed. The final pair's head0 PV interleaves into
the last step and the last two normalizations overlap the split-k output
projection drain, so only ~10us trails the last exp. Y is stored bf16
(upcast on host) to halve the output DMA.
"""

import numpy as np

HEADS = 8
DIM_HEAD = 64
SCALE = DIM_HEAD ** -0.5
B, NQ, DQ = 4, 2048, 512
NK, DC = 1024, 768
INNER = HEADS * DIM_HEAD  # 512
NQH = NQ // 2             # query rows per core
N_CORES = 8
P = 128

_PROG_CACHE = {}


def _build_program():
    import concourse.bacc as bacc
    import concourse.tile as tile
    from concourse import mybir
    from concourse.bass import ts, ds

    f32 = mybir.dt.float32
    f32r = mybir.dt.float32r
    bf16 = mybir.dt.bfloat16
    f8 = mybir.dt.float8e4
    Exp = mybir.ActivationFunctionType.Exp

    nc = bacc.Bacc(
        "TRN2",
        target_bir_lowering=False,
        debug=False,
        num_devices=N_CORES,
    )

    KQ = DQ // P      # 4  k-tiles for x-side contraction
    KC = DC // P      # 6  k-tiles for context-side contraction
    KI = INNER // P   # 4  k-tiles for inner-dim contraction
    NQT = NQH // P    # 8  query row tiles
    NKT = NK // P     # 8  key row tiles
    NCH = NQH // 512  # 2  nq chunks of 512

    # Wq/Wk host-tiled [m, p, ko, 128] so per-m loads are contiguous.
    xT_d = nc.dram_tensor("xT", [DQ, NQH], bf16, kind="ExternalInput")
    ctxT_d = nc.dram_tensor("ctxT", [DC, NK], bf16, kind="ExternalInput")
    Wq_d = nc.dram_tensor("Wq", [KQ, P, KQ, P], bf16, kind="ExternalInput")
    Wk_d = nc.dram_tensor("Wk", [KQ, P, KC, P], bf16, kind="ExternalInput")
    Wv_d = nc.dram_tensor("Wv", [P, KC, INNER], bf16, kind="ExternalInput")
    Wo_d = nc.dram_tensor("Wo", [P, KI, DQ], bf16, kind="ExternalInput")
    bo_d = nc.dram_tensor("bo", [DQ], f32, kind="ExternalInput")
    diag_d = nc.dram_tensor("diag2", [P, 128], bf16, kind="ExternalInput")
    Y_d = nc.dram_tensor("Y", [NQH, DQ], bf16, kind="ExternalOutput")

    with tile.TileContext(nc) as tc:
        with (
            tc.tile_pool(name="consts", bufs=1) as consts,
            tc.tile_pool(name="sc", bufs=2, space="PSUM") as scp,
            tc.tile_pool(name="pv", bufs=3, space="PSUM") as pvp,
            tc.tile_pool(name="mm", bufs=1, space="PSUM") as mmp,
            tc.tile_pool(name="ep", bufs=16) as ep,
            tc.tile_pool(name="dn", bufs=2) as dnp,
            tc.tile_pool(name="yp", bufs=2) as yp,
        ):
            Wk_sb = consts.tile([P, KQ, KC, P], bf16, tag="wk")
            Wq_sb = consts.tile([P, KQ, KQ, P], bf16, tag="wq")
            Wv_sb = consts.tile([P, KC, INNER], bf16, tag="wv")
            Wo_sb = consts.tile([P, KI, DQ], bf16, tag="wo")
            ctx_sb = consts.tile([P, KC, NK], bf16, tag="ctx")
            xT_sb = consts.tile([P, KQ, NQH], bf16, tag="x")
            bo_sb = consts.tile([P, DQ], f32, tag="bo")
            diag_sb = consts.tile([P, P], bf16, tag="diag")
            # two independent den-chain buffer sets alternating per step so
            # consecutive normalizations never serialize on WAR hazards.
            # r rows != {0, 64} must read as exact zeros for the mask-matmul
            # broadcast (0 * garbage-NaN would poison it): bf16 so memset
            # can clear them (and the broadcast matmul runs at bf16 rate).
            r_a = consts.tile([P, 512], bf16, tag="ra")
            r_b = consts.tile([P, 512], bf16, tag="rb2")
            d_a = consts.tile([P, 512], f32, tag="da")
            d_b = consts.tile([P, 512], f32, tag="db")

            ctx_src = ctxT_d.ap().rearrange("(ko p) n -> p ko n", p=P)
            xT_src = xT_d.ap().rearrange("(ko p) n -> p ko n", p=P)

            # ---- input DMA emission, in first-consumption order ----------
            # The three queues share the DMA engine pool (~60-100GB/s per
            # queue when all active); the first-needed chunks lead on every
            # queue. The issue (~0.7us engine time) is what occupies the
            # engine, transfers stream behind; scalar's issues all finish
            # before the first exp needs the ACT engine.
            def ctx_chunk(eng, k0, k1, c0, c1):
                eng.dma_start(
                    out=ctx_sb[:, k0:k1, c0:c1], in_=ctx_src[:, k0:k1, c0:c1]
                )

            def x_chunk(eng, k0, k1, c0, c1):
                eng.dma_start(
                    out=xT_sb[:, k0:k1, c0:c1], in_=xT_src[:, k0:k1, c0:c1]
                )

            nc.scalar.dma_start(out=Wk_sb[:, 0], in_=Wk_d.ap()[0])
            ctx_chunk(nc.sync, 0, 1, 0, 512)
            x_chunk(nc.gpsimd, 0, 2, 0, 512)
            ctx_chunk(nc.scalar, 4, 5, 0, 512)
            nc.sync.dma_start(out=Wq_sb[:, 0], in_=Wq_d.ap()[0])
            ctx_chunk(nc.gpsimd, 2, 3, 0, 512)
            ctx_chunk(nc.scalar, 5, 6, 0, 512)
            ctx_chunk(nc.sync, 1, 2, 0, 512)
            x_chunk(nc.gpsimd, 2, 4, 0, 512)
            nc.scalar.dma_start(out=Wq_sb[:, 1], in_=Wq_d.ap()[1])
            ctx_chunk(nc.sync, 3, 6, 512, 1024)
            ctx_chunk(nc.gpsimd, 3, 4, 0, 512)
            nc.scalar.dma_start(out=Wk_sb[:, 2], in_=Wk_d.ap()[2])
            nc.sync.dma_start(out=Wk_sb[:, 1], in_=Wk_d.ap()[1])
            ctx_chunk(nc.gpsimd, 0, 3, 512, 1024)
            nc.scalar.dma_start(out=Wq_sb[:, 2], in_=Wq_d.ap()[2])
            nc.sync.dma_start(
                out=Wv_sb[:, 0:3, :], in_=Wv_d.ap()[:, 0:3, :]
            )
            nc.scalar.dma_start(out=Wk_sb[:, 3], in_=Wk_d.ap()[3])
            nc.sync.dma_start(
                out=Wv_sb[:, 3:6, :], in_=Wv_d.ap()[:, 3:6, :]
            )
            nc.scalar.dma_start(out=Wq_sb[:, 3], in_=Wq_d.ap()[3])
            nc.gpsimd.dma_start(out=diag_sb, in_=diag_d.ap())
            x_chunk(nc.sync, 0, 4, 512, 1024)
            nc.sync.dma_start(out=Wo_sb, in_=Wo_d.ap())
            nc.gpsimd.dma_start(
                out=bo_sb, in_=bo_d.ap().unsqueeze(0).to_broadcast((P, DQ))
            )
            nc.gpsimd.memset(r_a, 0.0)
            nc.gpsimd.memset(r_b, 0.0)

            KT_sb = consts.tile([P, KI, NK], bf16, tag="kt")    # [i, nk]
            QT_sb = consts.tile([P, KI, NQH], bf16, tag="qt")   # [i, nq]
            # V in natural [nk, i] layout padded per head to 128 cols:
            # even head h: cols h*128+[0:64]=V_h, [64:128]=ones
            # odd  head h: cols h*128+[0:64]=ones, [64:128]=V_h
            V_sb = consts.tile([P, NKT, HEADS * P], bf16, tag="v")
            OT_sb = consts.tile([P, KI, NQH], bf16, tag="ot")   # [i, nq]

            for t in range(NKT):
                dv4 = V_sb[:, t, :].rearrange("p (j y) -> p j y", j=4)
                nc.gpsimd.memset(dv4[:, :, 64:192], 1.0)

            # ---- PE work units (emitted as schedule filler) ----
            def kp_unit(m, c):  # K projection: KT[:, m, c*512:...]
                psk = mmp.tile([P, 512], f32, tag="acc")
                for k in range(KC):
                    nc.tensor.matmul(
                        psk,
                        lhsT=Wk_sb[:, m, k, :],
                        rhs=ctx_sb[:, k, ds(c * 512, 512)],
                        start=(k == 0),
                        stop=(k == KC - 1),
                    )
                nc.vector.tensor_copy(KT_sb[:, m, ds(c * 512, 512)], psk)

            def qp_unit(m, c):  # Q projection: QT[:, m, c*512:...]
                psq = mmp.tile([P, 512], f32, tag="acc")
                for k in range(KQ):
                    nc.tensor.matmul(
                        psq,
                        lhsT=Wq_sb[:, m, k, :],
                        rhs=xT_sb[:, k, ds(c * 512, 512)],
                        start=(k == 0),
                        stop=(k == KQ - 1),
                    )
                nc.vector.tensor_copy(QT_sb[:, m, ds(c * 512, 512)], psq)

            def vp_unit(t):  # V projection tile t, scattered into head pads
                psv = mmp.tile([P, 512], f32, tag="acc")
                for k in range(KC):
                    nc.tensor.matmul(
                        psv,
                        lhsT=ctx_sb[:, k, ts(t, P)],
                        rhs=Wv_sb[:, k, :],
                        start=(k == 0),
                        stop=(k == KC - 1),
                    )
                pv4 = psv.rearrange("p (j x) -> p j x", j=4)
                dv4 = V_sb[:, t, :].rearrange("p (j y) -> p j y", j=4)
                nc.vector.tensor_copy(dv4[:, :, 0:64], pv4[:, :, 0:64])
                nc.vector.tensor_copy(dv4[:, :, 192:256], pv4[:, :, 64:128])

            def op_unit(m):  # output projection row tile m (full k)
                psy = mmp.tile([P, 512], f32, tag="acc")
                for k in range(KI):
                    nc.tensor.matmul(
                        psy,
                        lhsT=OT_sb[:, k, ts(m, P)],
                        rhs=Wo_sb[:, k, :],
                        start=(k == 0),
                        stop=(k == KI - 1),
                    )
                y_t = yp.tile([P, DQ], bf16, tag="y")
                nc.vector.tensor_tensor(y_t, psy, bo_sb, op=mybir.AluOpType.add)
                nc.sync.dma_start(out=Y_d.ap()[ts(m, P), :], in_=y_t)

            # split-k output projection for the last 4 row tiles: k=0..1 run
            # during the final step, k=2/k=3 drain between the last two
            # normalizations at the tail
            partials = {}

            def opk01_unit(m):
                psy = mmp.tile([P, 512], f32, tag="acc")
                for k in range(2):
                    nc.tensor.matmul(
                        psy,
                        lhsT=OT_sb[:, k, ts(m, P)],
                        rhs=Wo_sb[:, k, :],
                        start=(k == 0),
                        stop=(k == 1),
                    )
                part = yp.tile([P, 512], f32, tag="part", bufs=4)
                nc.vector.tensor_tensor(
                    part, psy, bo_sb, op=mybir.AluOpType.add
                )
                partials[m] = part

            def opk2_unit(m):
                psy = mmp.tile([P, 512], f32, tag="acc")
                nc.tensor.matmul(
                    psy, lhsT=OT_sb[:, 2, ts(m, P)], rhs=Wo_sb[:, 2, :],
                    start=True, stop=True,
                )
                part2 = yp.tile([P, 512], f32, tag="part2", bufs=4)
                nc.vector.tensor_tensor(
                    part2, psy, partials[m], op=mybir.AluOpType.add
                )
                partials[m] = part2

            def opk3_unit(m):
                psy = mmp.tile([P, 512], f32, tag="acc")
                nc.tensor.matmul(
                    psy, lhsT=OT_sb[:, 3, ts(m, P)], rhs=Wo_sb[:, 3, :],
                    start=True, stop=True,
                )
                y_t = yp.tile([P, DQ], bf16, tag="y")
                nc.vector.tensor_tensor(
                    y_t, psy, partials[m], op=mybir.AluOpType.add
                )
                nc.sync.dma_start(out=Y_d.ap()[ts(m, P), :], in_=y_t)

            def s_group(j, c, t):  # one nk-tile of scores for head pair j
                ps_g = scp.tile([P, 2, 512], f32, tag="s")
                e_g = ep.tile([P, 2, 512], bf16, tag="e")
                nc.tensor.matmul(
                    ps_g[:, 0, :],
                    lhsT=KT_sb[0:64, j, ts(t, P)],
                    rhs=QT_sb[0:64, j, ds(c * 512, 512)],
                    start=True, stop=True,
                )
                nc.tensor.matmul(
                    ps_g[:, 1, :],
                    lhsT=KT_sb[64:128, j, ts(t, P)],
                    rhs=QT_sb[64:128, j, ds(c * 512, 512)],
                    start=True, stop=True,
                )
                nc.scalar.activation(out=e_g, in_=ps_g, func=Exp, scale=SCALE)
                return e_g

            # ---- PV of a pair, emitted in per-slot chunks ---------------
            def new_pv(j, c, e_gs, parity):
                return {"j": j, "c": c, "e": e_gs, "par": parity,
                        "po": [None, None], "dr": None}

            def eng_copy(eng, out, in_):
                if hasattr(eng, "tensor_copy"):
                    eng.tensor_copy(out, in_)
                else:
                    eng.copy(out, in_)

            def pv_emit(st, ab, tts, cp=None, dq=None):
                """PV matmuls for head `2j+ab` over nk tiles tts; when the
                accumulation stops (tt==7), stage that head's den row and
                launch its half of the reciprocal chain. cp: engine for the
                PSUM->SBUF den-row copy (DVE default; ACT at the tail where
                it is idle). dq: DMA queue for chop/scatter (gpsimd default;
                sync at the tail)."""
                cp = cp or nc.vector
                dq = dq or nc.gpsimd
                d_sb = d_a if st["par"] == 0 else d_b
                r_sb = r_a if st["par"] == 0 else r_b
                h = 2 * st["j"] + ab
                if st["po"][ab] is None:
                    st["po"][ab] = pvp.tile(
                        [P, 512], f32, tag="po", name=f"po{ab}"
                    )
                po = st["po"][ab]
                for tt in tts:
                    nc.tensor.matmul(
                        po,
                        lhsT=V_sb[:, tt, ds(h * P, P)],
                        rhs=st["e"][tt][:, ab, :],
                        start=(tt == 0),
                        stop=(tt == NKT - 1),
                    )
                if tts[-1] == NKT - 1:
                    if ab == 0:
                        eng_copy(cp, d_sb[64:65, :], po[64:65, :])
                        st["dr"] = dnp.tile(
                            [64, 16], f32, tag="dr", name="dr"
                        )
                        dq.dma_start(
                            out=st["dr"][:, 0:8], in_=d_sb[64:65, :]
                        )
                    else:
                        dr = st["dr"]
                        eng_copy(cp, d_sb[0:1, :], po[0:1, :])
                        dq.dma_start(out=dr[:, 8:16], in_=d_sb[0:1, :])
                        rr = dnp.tile([64, 16], bf16, tag="rr")
                        with nc.allow_low_precision(
                            reason="1/den feeds a bf16 matmul"
                        ):
                            nc.vector.reciprocal(rr, dr)
                        dq.dma_start(out=r_sb[64:65, :], in_=rr[:, 0:8])
                        dq.dma_start(out=r_sb[0:1, :], in_=rr[:, 8:16])

            def make_finish(st, cp=None):
                cp = cp or nc.vector
                r_sb = r_a if st["par"] == 0 else r_b
                j, c, pos = st["j"], st["c"], st["po"]

                def finish():
                    ps_rb = mmp.tile([P, 512], f32, tag="acc")
                    nc.tensor.matmul(
                        ps_rb, lhsT=diag_sb, rhs=r_sb, start=True, stop=True
                    )
                    # DVE may read only one PSUM operand per instruction
                    rb_sb = yp.tile([P, 512], f32, tag="rb")
                    eng_copy(cp, rb_sb, ps_rb)
                    csl = ds(c * 512, 512)
                    nc.vector.tensor_tensor(
                        OT_sb[0:64, j, csl], pos[0][0:64, :], rb_sb[0:64, :],
                        op=mybir.AluOpType.mult,
                    )
                    nc.vector.tensor_tensor(
                        OT_sb[64:128, j, csl], pos[1][64:128, :],
                        rb_sb[64:128, :], op=mybir.AluOpType.mult,
                    )
                return finish

            # per-step slot fillers: {step: {slot: [unit, ...]}}
            sched = {
                0: {3: [lambda: kp_unit(0, 1)],
                    5: [lambda: kp_unit(1, 0)],
                    6: [lambda: qp_unit(1, 0), lambda: vp_unit(0)],
                    7: [lambda: kp_unit(1, 1), lambda: vp_unit(1),
                        lambda: vp_unit(2)]},
                1: {0: [lambda: vp_unit(3), lambda: vp_unit(4)],
                    1: [lambda: vp_unit(5), lambda: vp_unit(6)],
                    2: [lambda: vp_unit(7)],
                    5: [lambda: kp_unit(2, 0)],
                    6: [lambda: kp_unit(2, 1)],
                    7: [lambda: qp_unit(2, 0)]},
                2: {0: [lambda: kp_unit(3, 0)],
                    2: [lambda: kp_unit(3, 1)],
                    3: [lambda: qp_unit(3, 0)],
                    7: [lambda: qp_unit(0, 1)]},
                3: {0: [lambda: qp_unit(1, 1)],
                    2: [lambda: qp_unit(2, 1)]},
                4: {0: [lambda: qp_unit(3, 1)]},
                5: {5: [lambda: op_unit(0)], 7: [lambda: op_unit(1)]},
                6: {5: [lambda: op_unit(2)], 7: [lambda: op_unit(3)]},
                7: {7: [lambda: opk01_unit(4)]},
            }

            # ---- pre-loop + attention steps (c-outer) -------------------
            kp_unit(0, 0)
            qp_unit(0, 0)

            steps = [(j, c) for c in range(NCH) for j in range(HEADS // 2)]
            prev = None          # pv state awaiting its slot matmuls
            pending_norm = None  # broadcast + normalize of PV two steps back
            for i, (j, c) in enumerate(steps):
                e_gs = []
                cur = new_pv(j, c, e_gs, parity=i % 2)
                for t in range(NKT):
                    e_gs.append(s_group(j, c, t))
                    if t == 4 and pending_norm is not None:
                        pending_norm()
                        pending_norm = None
                    if prev is not None:
                        if t <= 3:
                            pv_emit(prev, 0, (2 * t, 2 * t + 1))
                        elif t == 4:
                            pv_emit(prev, 1, (0, 1, 2))
                        elif t == 5:
                            pv_emit(prev, 1, (3, 4, 5))
                        elif t == 6:
                            pv_emit(prev, 1, (6, 7))
                    if i == len(steps) - 1 and t >= 5:
                        # last pair's head0 PV interleaves into its own step
                        pv_emit(cur, 0, (2 * (t - 5), 2 * (t - 5) + 1))
                    for u in sched[i].get(t, []):
                        u()
                if prev is not None:
                    pending_norm = make_finish(prev)
                prev = cur

            # ---- tail: finish PV(3,1) + last two norms + split-k drain --
            # the ACT engine is idle after the last exp: route the den-row
            # and 1/den-broadcast copies through it, and the den DMAs
            # through the otherwise-idle sync queue.
            pv_emit(prev, 0, (6, 7), cp=nc.scalar, dq=nc.sync)
            opk01_unit(5)
            opk01_unit(6)
            opk01_unit(7)
            pending_norm()                 # norm of pair (2,1)
            pv_emit(prev, 1, tuple(range(NKT)), cp=nc.scalar, dq=nc.sync)
            for m in range(4, NQT):
                opk2_unit(m)
            fin = make_finish(prev, cp=nc.scalar)
            fin()                          # norm of pair (3,1)
            for m in range(4, NQT):
                opk3_unit(m)

    nc.finalize()
    return nc


def _get_program():
    if "nc" not in _PROG_CACHE:
        _PROG_CACHE["nc"] = _build_program()
    return _PROG_CACHE["nc"]


def _consts():
    import ml_dtypes
    # mask for the 1/den partition broadcast: contraction row 64 carries the
    # even head's reciprocal (-> out partitions 0:64 where its O rows live),
    # row 0 carries the odd head's (-> out partitions 64:128)
    diag = np.zeros((128, 128), dtype=ml_dtypes.bfloat16)
    diag[64, 0:64] = 1.0
    diag[0, 64:128] = 1.0
    return diag


def _prep_shared(Wq, Wk, Wv, Wo, bo):
    """Host-side weight tiling shared by all cores."""
    import ml_dtypes
    bf = ml_dtypes.bfloat16

    def _tile_w(w):  # [(ko p), n] -> [p, ko, n] contiguous bf16
        w = np.asarray(w, dtype=np.float32).astype(bf)
        ko = w.shape[0] // P
        return np.ascontiguousarray(
            w.reshape(ko, P, w.shape[1]).transpose(1, 0, 2)
        )

    def _tile_w_m(w):  # [(ko p), (m 128)] -> [m, p, ko, 128] contiguous
        w = np.asarray(w, dtype=np.float32).astype(bf)
        ko = w.shape[0] // P
        m = w.shape[1] // P
        return np.ascontiguousarray(
            w.reshape(ko, P, m, P).transpose(2, 1, 0, 3)
        )

    Wqb = _tile_w_m(Wq)
    Wkb = _tile_w_m(Wk)
    Wvb = _tile_w(Wv)
    Wob = _tile_w(Wo)
    bob = np.ascontiguousarray(np.asarray(bo, dtype=np.float32))
    return Wqb, Wkb, Wvb, Wob, bob


def _make_in_maps(x, context, Wq, Wk, Wv, Wo, bo):
    import ml_dtypes
    bf = ml_dtypes.bfloat16

    x = np.asarray(x, dtype=np.float32)
    context = np.asarray(context, dtype=np.float32)
    Wqb, Wkb, Wvb, Wob, bob = _prep_shared(Wq, Wk, Wv, Wo, bo)
    diag = _consts()

    in_maps = []
    for core in range(N_CORES):
        b, half = divmod(core, 2)
        xs = np.ascontiguousarray(
            x[b, half * NQH:(half + 1) * NQH, :].T.astype(bf)
        )
        cs = np.ascontiguousarray(context[b].T.astype(bf))
        in_maps.append(
            {"xT": xs, "ctxT": cs, "Wq": Wqb, "Wk": Wkb, "Wv": Wvb,
             "Wo": Wob, "bo": bob, "diag2": diag}
        )
    return in_maps


def kernel(x, context, Wq, Wk, Wv, Wo, bo, **_unused):
    from concourse.bass_utils import run_bass_kernel_spmd

    nc = _get_program()
    in_maps = _make_in_maps(x, context, Wq, Wk, Wv, Wo, bo)
    res = run_bass_kernel_spmd(nc, in_maps, core_ids=list(range(N_CORES)))

    out = np.empty((B, NQ, DQ), np.float32)
    for core in range(N_CORES):
        b, half = divmod(core, 2)
        out[b, half * NQH:(half + 1) * NQH, :] = (
            res.results[core]["Y"].astype(np.float32)
        )
    return out


# revision 17
# speedup vs baseline: 1.1445x; 1.0472x over previous
"""CrossAttention kernel for 8 Trainium2 NeuronCores.

Sharding: batch (4) x query-row-half (2) -> 8 shards, one per core. Each core
computes the full cross-attention for its 1024 query rows of one batch:
Q/K/V projections, 8 heads of attention, and the output projection. K/V
projections are recomputed by both cores sharing a batch (20% extra flops)
in exchange for zero collectives and a pure-SPMD single NEFF.

Layout trick: x and context are transposed (and cast to bf16) on the host so
the contraction dim lands on SBUF partitions with contiguous DMAs; all device
matmuls run without on-chip transposes:
  QT = Wq.T @ xT      (i on partitions)     KT = Wk.T @ ctxT
  V  = ctxT.T @ Wv    (natural [nk, i])
  ST_h = KT_h @ QT_h  ([nk, nq], K=64, head pairs in PE row groups)
  P = exp(ST * scale) (no max-subtraction; logits are ~N(0,1), safe range)
  O^T_h | den_h = [V_h | ones].T @ P  (denominator rides free in the M dim)
  Y = (O^T/den).T @ Wo + bo

Schedule (v4): engine queues execute in order, so the emission order IS the
schedule. Each of a step's 8 score groups is followed immediately by PV
matmuls of the previous step's pair (head0 2/slot in slots 0-3, head1
3/3/2 in slots 5-7) plus at most ~1 projection unit, so the ACT engine's
exp stream never waits more than a slot. The softmax denominator chain
(stage den row -> chop DMA -> reciprocal -> scatter DMA) launches per head
as soon as that head's accumulation stops; the 1/den broadcast + normalize
runs at slot 4 two steps later, which also bounds PV-accumulator lifetime
so 3 PSUM banks suffice (scores 4 + PV 3 + scratch 1 = 8). Input DMAs are
chunked in first-consumption order across the three queues (~100GB/s
each): scalar carries the small weight tiles and finishes before the
first exp, sync and gpsimd carry the bulk. V-pad ones blocks are memset
on gpsimd instead of DMAed. The final pair's head0 PV interleaves into
the last step and the last two normalizations overlap the split-k output
projection drain, so only ~10us trails the last exp. Y is stored bf16
(upcast on host) to halve the output DMA.
"""

import numpy as np

HEADS = 8
DIM_HEAD = 64
SCALE = DIM_HEAD ** -0.5
B, NQ, DQ = 4, 2048, 512
NK, DC = 1024, 768
INNER = HEADS * DIM_HEAD  # 512
NQH = NQ // 2             # query rows per core
N_CORES = 8
P = 128

_PROG_CACHE = {}


def _build_program():
    import concourse.bacc as bacc
    import concourse.tile as tile
    from concourse import mybir
    from concourse.bass import ts, ds

    f32 = mybir.dt.float32
    f32r = mybir.dt.float32r
    bf16 = mybir.dt.bfloat16
    f8 = mybir.dt.float8e4
    Exp = mybir.ActivationFunctionType.Exp

    nc = bacc.Bacc(
        "TRN2",
        target_bir_lowering=False,
        debug=False,
        num_devices=N_CORES,
    )

    KQ = DQ // P      # 4  k-tiles for x-side contraction
    KC = DC // P      # 6  k-tiles for context-side contraction
    KI = INNER // P   # 4  k-tiles for inner-dim contraction
    NQT = NQH // P    # 8  query row tiles
    NKT = NK // P     # 8  key row tiles
    NCH = NQH // 512  # 2  nq chunks of 512

    # Wq/Wk host-tiled [m, p, ko, 128] so per-m loads are contiguous.
    xT_d = nc.dram_tensor("xT", [DQ, NQH], bf16, kind="ExternalInput")
    ctxT_d = nc.dram_tensor("ctxT", [DC, NK], bf16, kind="ExternalInput")
    Wq_d = nc.dram_tensor("Wq", [KQ, P, KQ, P], bf16, kind="ExternalInput")
    Wk_d = nc.dram_tensor("Wk", [KQ, P, KC, P], bf16, kind="ExternalInput")
    Wv_d = nc.dram_tensor("Wv", [P, KC, INNER], bf16, kind="ExternalInput")
    Wo_d = nc.dram_tensor("Wo", [P, KI, DQ], bf16, kind="ExternalInput")
    bo_d = nc.dram_tensor("bo", [DQ], f32, kind="ExternalInput")
    diag_d = nc.dram_tensor("diag2", [P, 128], bf16, kind="ExternalInput")
    Y_d = nc.dram_tensor("Y", [NQH, DQ], bf16, kind="ExternalOutput")

    with tile.TileContext(nc) as tc:
        with (
            tc.tile_pool(name="consts", bufs=1) as consts,
            tc.tile_pool(name="sc", bufs=2, space="PSUM") as scp,
            tc.tile_pool(name="pv", bufs=3, space="PSUM") as pvp,
            tc.tile_pool(name="mm", bufs=1, space="PSUM") as mmp,
            tc.tile_pool(name="ep", bufs=16) as ep,
            tc.tile_pool(name="dn", bufs=2) as dnp,
            tc.tile_pool(name="yp", bufs=2) as yp,
        ):
            Wk_sb = consts.tile([P, KQ, KC, P], bf16, tag="wk")
            Wq_sb = consts.tile([P, KQ, KQ, P], bf16, tag="wq")
            Wv_sb = consts.tile([P, KC, INNER], bf16, tag="wv")
            Wo_sb = consts.tile([P, KI, DQ], bf16, tag="wo")
            ctx_sb = consts.tile([P, KC, NK], bf16, tag="ctx")
            xT_sb = consts.tile([P, KQ, NQH], bf16, tag="x")
            bo_sb = consts.tile([P, DQ], f32, tag="bo")
            diag_sb = consts.tile([P, P], bf16, tag="diag")
            # two independent den-chain buffer sets alternating per step so
            # consecutive normalizations never serialize on WAR hazards.
            # r rows != {0, 64} must read as exact zeros for the mask-matmul
            # broadcast (0 * garbage-NaN would poison it): bf16 so memset
            # can clear them (and the broadcast matmul runs at bf16 rate).
            r_a = consts.tile([P, 512], bf16, tag="ra")
            r_b = consts.tile([P, 512], bf16, tag="rb2")
            d_a = consts.tile([P, 512], f32, tag="da")
            d_b = consts.tile([P, 512], f32, tag="db")

            ctx_src = ctxT_d.ap().rearrange("(ko p) n -> p ko n", p=P)
            xT_src = xT_d.ap().rearrange("(ko p) n -> p ko n", p=P)

            # ---- input DMA emission, in first-consumption order ----------
            # The three queues share the DMA engine pool (~60-100GB/s per
            # queue when all active); the first-needed chunks lead on every
            # queue. The issue (~0.7us engine time) is what occupies the
            # engine, transfers stream behind; scalar's issues all finish
            # before the first exp needs the ACT engine.
            def ctx_chunk(eng, k0, k1, c0, c1):
                eng.dma_start(
                    out=ctx_sb[:, k0:k1, c0:c1], in_=ctx_src[:, k0:k1, c0:c1]
                )

            def x_chunk(eng, k0, k1, c0, c1):
                eng.dma_start(
                    out=xT_sb[:, k0:k1, c0:c1], in_=xT_src[:, k0:k1, c0:c1]
                )

            nc.scalar.dma_start(out=Wk_sb[:, 0], in_=Wk_d.ap()[0])
            ctx_chunk(nc.sync, 0, 1, 0, 512)
            x_chunk(nc.gpsimd, 0, 2, 0, 512)
            ctx_chunk(nc.scalar, 4, 5, 0, 512)
            nc.sync.dma_start(out=Wq_sb[:, 0], in_=Wq_d.ap()[0])
            ctx_chunk(nc.gpsimd, 2, 3, 0, 512)
            ctx_chunk(nc.scalar, 5, 6, 0, 512)
            ctx_chunk(nc.sync, 1, 2, 0, 512)
            x_chunk(nc.gpsimd, 2, 4, 0, 512)
            nc.scalar.dma_start(out=Wq_sb[:, 1], in_=Wq_d.ap()[1])
            ctx_chunk(nc.sync, 3, 6, 512, 1024)
            ctx_chunk(nc.gpsimd, 3, 4, 0, 512)
            nc.scalar.dma_start(out=Wk_sb[:, 2], in_=Wk_d.ap()[2])
            nc.sync.dma_start(out=Wk_sb[:, 1], in_=Wk_d.ap()[1])
            ctx_chunk(nc.gpsimd, 0, 3, 512, 1024)
            nc.scalar.dma_start(
                out=Wv_sb[:, 0:3, :], in_=Wv_d.ap()[:, 0:3, :]
            )
            nc.scalar.dma_start(out=Wq_sb[:, 2], in_=Wq_d.ap()[2])
            nc.scalar.dma_start(
                out=Wv_sb[:, 3:6, :], in_=Wv_d.ap()[:, 3:6, :]
            )
            nc.scalar.dma_start(out=Wk_sb[:, 3], in_=Wk_d.ap()[3])
            nc.scalar.dma_start(out=Wq_sb[:, 3], in_=Wq_d.ap()[3])
            nc.gpsimd.dma_start(out=diag_sb, in_=diag_d.ap())
            x_chunk(nc.sync, 0, 4, 512, 1024)
            nc.sync.dma_start(out=Wo_sb, in_=Wo_d.ap())
            nc.gpsimd.dma_start(
                out=bo_sb, in_=bo_d.ap().unsqueeze(0).to_broadcast((P, DQ))
            )
            nc.gpsimd.memset(r_a, 0.0)
            nc.gpsimd.memset(r_b, 0.0)

            KT_sb = consts.tile([P, KI, NK], bf16, tag="kt")    # [i, nk]
            QT_sb = consts.tile([P, KI, NQH], bf16, tag="qt")   # [i, nq]
            # V in natural [nk, i] layout padded per head to 128 cols:
            # even head h: cols h*128+[0:64]=V_h, [64:128]=ones
            # odd  head h: cols h*128+[0:64]=ones, [64:128]=V_h
            V_sb = consts.tile([P, NKT, HEADS * P], bf16, tag="v")
            OT_sb = consts.tile([P, KI, NQH], bf16, tag="ot")   # [i, nq]

            for t in range(NKT):
                dv4 = V_sb[:, t, :].rearrange("p (j y) -> p j y", j=4)
                nc.gpsimd.memset(dv4[:, :, 64:192], 1.0)

            # ---- PE work units (emitted as schedule filler) ----
            def kp_unit(m, c):  # K projection: KT[:, m, c*512:...]
                psk = mmp.tile([P, 512], f32, tag="acc")
                for k in range(KC):
                    nc.tensor.matmul(
                        psk,
                        lhsT=Wk_sb[:, m, k, :],
                        rhs=ctx_sb[:, k, ds(c * 512, 512)],
                        start=(k == 0),
                        stop=(k == KC - 1),
                    )
                nc.vector.tensor_copy(KT_sb[:, m, ds(c * 512, 512)], psk)

            def qp_unit(m, c):  # Q projection: QT[:, m, c*512:...]
                psq = mmp.tile([P, 512], f32, tag="acc")
                for k in range(KQ):
                    nc.tensor.matmul(
                        psq,
                        lhsT=Wq_sb[:, m, k, :],
                        rhs=xT_sb[:, k, ds(c * 512, 512)],
                        start=(k == 0),
                        stop=(k == KQ - 1),
                    )
                nc.vector.tensor_copy(QT_sb[:, m, ds(c * 512, 512)], psq)

            def vp_unit(t):  # V projection tile t, scattered into head pads
                psv = mmp.tile([P, 512], f32, tag="acc")
                for k in range(KC):
                    nc.tensor.matmul(
                        psv,
                        lhsT=ctx_sb[:, k, ts(t, P)],
                        rhs=Wv_sb[:, k, :],
                        start=(k == 0),
                        stop=(k == KC - 1),
                    )
                pv4 = psv.rearrange("p (j x) -> p j x", j=4)
                dv4 = V_sb[:, t, :].rearrange("p (j y) -> p j y", j=4)
                nc.vector.tensor_copy(dv4[:, :, 0:64], pv4[:, :, 0:64])
                nc.vector.tensor_copy(dv4[:, :, 192:256], pv4[:, :, 64:128])

            def op_unit(m):  # output projection row tile m (full k)
                psy = mmp.tile([P, 512], f32, tag="acc")
                for k in range(KI):
                    nc.tensor.matmul(
                        psy,
                        lhsT=OT_sb[:, k, ts(m, P)],
                        rhs=Wo_sb[:, k, :],
                        start=(k == 0),
                        stop=(k == KI - 1),
                    )
                y_t = yp.tile([P, DQ], bf16, tag="y")
                nc.vector.tensor_tensor(y_t, psy, bo_sb, op=mybir.AluOpType.add)
                nc.sync.dma_start(out=Y_d.ap()[ts(m, P), :], in_=y_t)

            # split-k output projection for the last 4 row tiles: k=0..1 run
            # during the final step, k=2/k=3 drain between the last two
            # normalizations at the tail
            partials = {}

            def opk01_unit(m):
                psy = mmp.tile([P, 512], f32, tag="acc")
                for k in range(2):
                    nc.tensor.matmul(
                        psy,
                        lhsT=OT_sb[:, k, ts(m, P)],
                        rhs=Wo_sb[:, k, :],
                        start=(k == 0),
                        stop=(k == 1),
                    )
                part = yp.tile([P, 512], f32, tag="part", bufs=4)
                nc.vector.tensor_tensor(
                    part, psy, bo_sb, op=mybir.AluOpType.add
                )
                partials[m] = part

            def opk2_unit(m):
                psy = mmp.tile([P, 512], f32, tag="acc")
                nc.tensor.matmul(
                    psy, lhsT=OT_sb[:, 2, ts(m, P)], rhs=Wo_sb[:, 2, :],
                    start=True, stop=True,
                )
                part2 = yp.tile([P, 512], f32, tag="part2", bufs=4)
                nc.vector.tensor_tensor(
                    part2, psy, partials[m], op=mybir.AluOpType.add
                )
                partials[m] = part2

            def opk3_unit(m):
                psy = mmp.tile([P, 512], f32, tag="acc")
                nc.tensor.matmul(
                    psy, lhsT=OT_sb[:, 3, ts(m, P)], rhs=Wo_sb[:, 3, :],
                    start=True, stop=True,
                )
                y_t = yp.tile([P, DQ], bf16, tag="y")
                nc.vector.tensor_tensor(
                    y_t, psy, partials[m], op=mybir.AluOpType.add
                )
                nc.sync.dma_start(out=Y_d.ap()[ts(m, P), :], in_=y_t)

            def s_group(j, c, t):  # one nk-tile of scores for head pair j
                ps_g = scp.tile([P, 2, 512], f32, tag="s")
                e_g = ep.tile([P, 2, 512], bf16, tag="e")
                nc.tensor.matmul(
                    ps_g[:, 0, :],
                    lhsT=KT_sb[0:64, j, ts(t, P)],
                    rhs=QT_sb[0:64, j, ds(c * 512, 512)],
                    start=True, stop=True,
                )
                nc.tensor.matmul(
                    ps_g[:, 1, :],
                    lhsT=KT_sb[64:128, j, ts(t, P)],
                    rhs=QT_sb[64:128, j, ds(c * 512, 512)],
                    start=True, stop=True,
                )
                nc.scalar.activation(out=e_g, in_=ps_g, func=Exp, scale=SCALE)
                return e_g

            # ---- PV of a pair, emitted in per-slot chunks ---------------
            def new_pv(j, c, e_gs, parity):
                return {"j": j, "c": c, "e": e_gs, "par": parity,
                        "po": [None, None], "dr": None}

            def eng_copy(eng, out, in_):
                if hasattr(eng, "tensor_copy"):
                    eng.tensor_copy(out, in_)
                else:
                    eng.copy(out, in_)

            def pv_emit(st, ab, tts, cp=None, dq=None):
                """PV matmuls for head `2j+ab` over nk tiles tts; when the
                accumulation stops (tt==7), stage that head's den row and
                launch its half of the reciprocal chain. cp: engine for the
                PSUM->SBUF den-row copy (DVE default; ACT at the tail where
                it is idle). dq: DMA queue for chop/scatter (gpsimd default;
                sync at the tail)."""
                cp = cp or nc.vector
                dq = dq or nc.gpsimd
                d_sb = d_a if st["par"] == 0 else d_b
                r_sb = r_a if st["par"] == 0 else r_b
                h = 2 * st["j"] + ab
                if st["po"][ab] is None:
                    st["po"][ab] = pvp.tile(
                        [P, 512], f32, tag="po", name=f"po{ab}"
                    )
                po = st["po"][ab]
                for tt in tts:
                    nc.tensor.matmul(
                        po,
                        lhsT=V_sb[:, tt, ds(h * P, P)],
                        rhs=st["e"][tt][:, ab, :],
                        start=(tt == 0),
                        stop=(tt == NKT - 1),
                    )
                if tts[-1] == NKT - 1:
                    if ab == 0:
                        eng_copy(cp, d_sb[64:65, :], po[64:65, :])
                        st["dr"] = dnp.tile(
                            [64, 16], f32, tag="dr", name="dr"
                        )
                        dq.dma_start(
                            out=st["dr"][:, 0:8], in_=d_sb[64:65, :]
                        )
                    else:
                        dr = st["dr"]
                        eng_copy(cp, d_sb[0:1, :], po[0:1, :])
                        dq.dma_start(out=dr[:, 8:16], in_=d_sb[0:1, :])
                        rr = dnp.tile([64, 16], bf16, tag="rr")
                        with nc.allow_low_precision(
                            reason="1/den feeds a bf16 matmul"
                        ):
                            nc.vector.reciprocal(rr, dr)
                        dq.dma_start(out=r_sb[64:65, :], in_=rr[:, 0:8])
                        dq.dma_start(out=r_sb[0:1, :], in_=rr[:, 8:16])

            def make_finish(st, cp=None):
                cp = cp or nc.vector
                r_sb = r_a if st["par"] == 0 else r_b
                j, c, pos = st["j"], st["c"], st["po"]

                def finish():
                    ps_rb = mmp.tile([P, 512], f32, tag="acc")
                    nc.tensor.matmul(
                        ps_rb, lhsT=diag_sb, rhs=r_sb, start=True, stop=True
                    )
                    # DVE may read only one PSUM operand per instruction
                    rb_sb = yp.tile([P, 512], f32, tag="rb")
                    eng_copy(cp, rb_sb, ps_rb)
                    csl = ds(c * 512, 512)
                    nc.vector.tensor_tensor(
                        OT_sb[0:64, j, csl], pos[0][0:64, :], rb_sb[0:64, :],
                        op=mybir.AluOpType.mult,
                    )
                    nc.vector.tensor_tensor(
                        OT_sb[64:128, j, csl], pos[1][64:128, :],
                        rb_sb[64:128, :], op=mybir.AluOpType.mult,
                    )
                return finish

            # per-step slot fillers: {step: {slot: [unit, ...]}}
            sched = {
                0: {3: [lambda: kp_unit(0, 1)],
                    5: [lambda: kp_unit(1, 0)],
                    6: [lambda: qp_unit(1, 0), lambda: vp_unit(0),
                        lambda: vp_unit(1)],
                    7: [lambda: kp_unit(1, 1), lambda: vp_unit(2),
                        lambda: vp_unit(3), lambda: vp_unit(4)]},
                1: {0: [lambda: vp_unit(5)],
                    1: [lambda: vp_unit(6)],
                    2: [lambda: vp_unit(7)],
                    3: [lambda: qp_unit(2, 0)],
                    4: [lambda: kp_unit(2, 0)],
                    5: [lambda: kp_unit(2, 1)]},
                2: {0: [lambda: kp_unit(3, 0)],
                    2: [lambda: kp_unit(3, 1)],
                    3: [lambda: qp_unit(3, 0)],
                    7: [lambda: qp_unit(0, 1)]},
                3: {0: [lambda: qp_unit(1, 1)],
                    2: [lambda: qp_unit(2, 1)]},
                4: {0: [lambda: qp_unit(3, 1)]},
                5: {5: [lambda: op_unit(0)], 7: [lambda: op_unit(1)]},
                6: {5: [lambda: op_unit(2)], 7: [lambda: op_unit(3)]},
                7: {7: [lambda: opk01_unit(4)]},
            }

            # ---- pre-loop + attention steps (c-outer) -------------------
            qp_unit(0, 0)
            kp_unit(0, 0)

            steps = [(j, c) for c in range(NCH) for j in range(HEADS // 2)]
            prev = None          # pv state awaiting its slot matmuls
            pending_norm = None  # broadcast + normalize of PV two steps back
            for i, (j, c) in enumerate(steps):
                e_gs = []
                cur = new_pv(j, c, e_gs, parity=i % 2)
                for t in range(NKT):
                    e_gs.append(s_group(j, c, t))
                    if t == 4 and pending_norm is not None:
                        pending_norm()
                        pending_norm = None
                    if prev is not None:
                        if t <= 3:
                            pv_emit(prev, 0, (2 * t, 2 * t + 1))
                        elif t == 4:
                            pv_emit(prev, 1, (0, 1, 2))
                        elif t == 5:
                            pv_emit(prev, 1, (3, 4, 5))
                        elif t == 6:
                            pv_emit(prev, 1, (6, 7))
                    if i == len(steps) - 1 and t >= 5:
                        # last pair's head0 PV interleaves into its own step
                        pv_emit(cur, 0, (2 * (t - 5), 2 * (t - 5) + 1))
                    for u in sched[i].get(t, []):
                        u()
                if prev is not None:
                    pending_norm = make_finish(prev)
                prev = cur

            # ---- tail: finish PV(3,1) + last two norms + split-k drain --
            # the ACT engine is idle after the last exp: route the den-row
            # and 1/den-broadcast copies through it, and the den DMAs
            # through the otherwise-idle sync queue.
            pv_emit(prev, 0, (6, 7), cp=nc.scalar, dq=nc.sync)
            opk01_unit(5)
            opk01_unit(6)
            opk01_unit(7)
            pending_norm()                 # norm of pair (2,1)
            pv_emit(prev, 1, tuple(range(NKT)), cp=nc.scalar, dq=nc.sync)
            for m in range(4, NQT):
                opk2_unit(m)
            fin = make_finish(prev, cp=nc.scalar)
            fin()                          # norm of pair (3,1)
            for m in range(4, NQT):
                opk3_unit(m)

    nc.finalize()
    return nc


def _get_program():
    if "nc" not in _PROG_CACHE:
        _PROG_CACHE["nc"] = _build_program()
    return _PROG_CACHE["nc"]


def _consts():
    import ml_dtypes
    # mask for the 1/den partition broadcast: contraction row 64 carries the
    # even head's reciprocal (-> out partitions 0:64 where its O rows live),
    # row 0 carries the odd head's (-> out partitions 64:128)
    diag = np.zeros((128, 128), dtype=ml_dtypes.bfloat16)
    diag[64, 0:64] = 1.0
    diag[0, 64:128] = 1.0
    return diag


def _prep_shared(Wq, Wk, Wv, Wo, bo):
    """Host-side weight tiling shared by all cores."""
    import ml_dtypes
    bf = ml_dtypes.bfloat16

    def _tile_w(w):  # [(ko p), n] -> [p, ko, n] contiguous bf16
        w = np.asarray(w, dtype=np.float32).astype(bf)
        ko = w.shape[0] // P
        return np.ascontiguousarray(
            w.reshape(ko, P, w.shape[1]).transpose(1, 0, 2)
        )

    def _tile_w_m(w):  # [(ko p), (m 128)] -> [m, p, ko, 128] contiguous
        w = np.asarray(w, dtype=np.float32).astype(bf)
        ko = w.shape[0] // P
        m = w.shape[1] // P
        return np.ascontiguousarray(
            w.reshape(ko, P, m, P).transpose(2, 1, 0, 3)
        )

    Wqb = _tile_w_m(Wq)
    Wkb = _tile_w_m(Wk)
    Wvb = _tile_w(Wv)
    Wob = _tile_w(Wo)
    bob = np.ascontiguousarray(np.asarray(bo, dtype=np.float32))
    return Wqb, Wkb, Wvb, Wob, bob


def _make_in_maps(x, context, Wq, Wk, Wv, Wo, bo):
    import ml_dtypes
    bf = ml_dtypes.bfloat16

    x = np.asarray(x, dtype=np.float32)
    context = np.asarray(context, dtype=np.float32)
    Wqb, Wkb, Wvb, Wob, bob = _prep_shared(Wq, Wk, Wv, Wo, bo)
    diag = _consts()

    in_maps = []
    for core in range(N_CORES):
        b, half = divmod(core, 2)
        xs = np.ascontiguousarray(
            x[b, half * NQH:(half + 1) * NQH, :].T.astype(bf)
        )
        cs = np.ascontiguousarray(context[b].T.astype(bf))
        in_maps.append(
            {"xT": xs, "ctxT": cs, "Wq": Wqb, "Wk": Wkb, "Wv": Wvb,
             "Wo": Wob, "bo": bob, "diag2": diag}
        )
    return in_maps


def kernel(x, context, Wq, Wk, Wv, Wo, bo, **_unused):
    from concourse.bass_utils import run_bass_kernel_spmd

    nc = _get_program()
    in_maps = _make_in_maps(x, context, Wq, Wk, Wv, Wo, bo)
    res = run_bass_kernel_spmd(nc, in_maps, core_ids=list(range(N_CORES)))

    out = np.empty((B, NQ, DQ), np.float32)
    for core in range(N_CORES):
        b, half = divmod(core, 2)
        out[b, half * NQH:(half + 1) * NQH, :] = (
            res.results[core]["Y"].astype(np.float32)
        )
    return out
